# revision 18
# baseline (speedup 1.0000x reference)
"""Trainium2 Bass kernel for nn_MixedHeadsV2 (mixed-head causal attention).

Full inputs in, full output out. Sharding: 8 cores = 4 batches x 2 head-groups.
Each core handles one batch and 4 of the 8 base heads: even cores heads
{0,1,4,5}, odd cores {2,3,6,7}. Heads 0-3 ("heavy") have effective head size
128; heads 4-7 ("light") have effective head size 64 (their mixed weight rows
64:128 are exactly zero), so the two light heads are packed into one 128-wide
tensor for projections and use 64-partition (K=64) score matmuls.

v2 engine plan (per core, Tile-scheduled, all instruction streams
software-pipelined so ACT/exp never starves and PE never idles):
  PE:   fp32 x-transposes, bf16 W-transposes, projections, scores, AV.
        AV of t-chunk tj is emitted during scores of chunk tj+1 (delay-1
        software pipeline); projections of the next unit and the x/W prep
        fill remaining PE slack via a cost-budgeted filler queue.
  ACT:  exp ONLY (scale folded), groups of 3 score tiles [128,1536] from a
        2-buf x 3-bank PSUM pool.
  DVE:  PSUM->SBUF glue (x consolidate+cast, qk casts, v copies, AV copies),
        fused 2-block causal tri masks.
  Pool: normalize_recip (out = av/denom, SBUF-side), vtile ones init,
        weight-DMA issue, affine_select consts.
  Sync: x loads + output stores.
"""
import sys

for p in ("/opt/trn_rl_repo",):
    if p not in sys.path:
        sys.path.append(p)

import numpy as np

import concourse.bass as bass
import concourse.tile as tile
from concourse import bacc, mybir
from concourse.bass_utils import run_bass_kernel_spmd

FP32 = mybir.dt.float32
BF16 = mybir.dt.bfloat16
AF = mybir.ActivationFunctionType
ALU = mybir.AluOpType

T = 2048
C = 512
HS = 128          # heavy head size (= padded head size)
NT128 = T // 128  # 16
NT512 = T // 512  # 4
NCC = C // 128    # 4
SCALE = float(1.0 / np.sqrt(128.0))
SGRP = 3          # score tiles (512 wide) per exp group / PSUM banks per buf

_CACHE = {}


def _build():
    nc = bacc.Bacc("TRN2", target_bir_lowering=False, debug=False, num_devices=8)
    x_d = nc.dram_tensor("x", [T, C], FP32, kind="ExternalInput")
    w_d = nc.dram_tensor("w", [4, 1], FP32, kind="ExternalInput")
    bq_d = nc.dram_tensor("bq", [4, HS, C], FP32, kind="ExternalInput")
    bk_d = nc.dram_tensor("bk", [4, HS, C], FP32, kind="ExternalInput")
    bv_d = nc.dram_tensor("bv", [4, HS, C], FP32, kind="ExternalInput")
    out_d = nc.dram_tensor("out", [T, 4 * HS], FP32, kind="ExternalOutput")

    with tile.TileContext(nc) as tc:
        _emit(nc, tc, x_d, w_d, bq_d, bk_d, bv_d, out_d)
    nc.compile()
    return nc


def _emit(nc, tc, x_d, w_d, bq_d, bk_d, bv_d, out_d):
    from contextlib import ExitStack

    ctx = ExitStack()
    with ctx:
        # ---- persistent SBUF pools ----
        const_p = ctx.enter_context(tc.tile_pool(name="const", bufs=1))
        wall_p = ctx.enter_context(tc.tile_pool(name="wall", bufs=1))
        wts_p = ctx.enter_context(tc.tile_pool(name="wts", bufs=1))
        xall_p = ctx.enter_context(tc.tile_pool(name="xall", bufs=1))
        xt_p = ctx.enter_context(tc.tile_pool(name="xt", bufs=1))
        qk_p = ctx.enter_context(tc.tile_pool(name="qk", bufs=1))
        v_p = ctx.enter_context(tc.tile_pool(name="v", bufs=1))
        pt_p = ctx.enter_context(tc.tile_pool(name="pt", bufs=1))
        osb_p = ctx.enter_context(tc.tile_pool(name="osb", bufs=8))
        o_p = ctx.enter_context(tc.tile_pool(name="o", bufs=4))
        stage_p = ctx.enter_context(tc.tile_pool(name="stage", bufs=2))
        # ---- PSUM: 2 bufs x 3 banks for score groups + 2 x 1 bank shared
        sps = ctx.enter_context(tc.tile_pool(name="sps", bufs=2, space="PSUM"))
        ps = ctx.enter_context(tc.tile_pool(name="ps", bufs=2, space="PSUM"))

        # ================= constants (first: Pool/DVE free, no DMA deps) ===
        ones_b = const_p.tile([128, C], BF16, tag="ones_b")
        nc.vector.memset(ones_b[:], 1.0)
        ident_b = const_p.tile([128, 128], BF16, tag="ident_b")
        nc.gpsimd.affine_select(
            ident_b[:], ones_b[:, 0:128], pattern=[[1, 128]],
            compare_op=ALU.is_equal, fill=0.0, base=0, channel_multiplier=-1)
        ident_f = const_p.tile([128, 128], FP32, tag="ident_f")
        nc.vector.tensor_copy(ident_f[:], ident_b[:])
        # causal triangle x4: tri4[:, r*128+t] = (t >= s) for r = 0..3
        tri4 = const_p.tile([128, 512], BF16, tag="tri4")
        for r in range(4):
            nc.gpsimd.affine_select(
                tri4[:, r * 128:(r + 1) * 128], ones_b[:, 0:128],
                pattern=[[1, 128]], compare_op=ALU.is_ge, fill=0.0, base=0,
                channel_multiplier=-1)
        tri3 = tri4[:].rearrange("p (r q) -> p r q", r=4)
        # start the PE streaming immediately (HAM clock-gate warm-up)
        for _ in range(3):
            sp0 = ps.tile([128, 512], FP32, name="ps", tag="ps")
            nc.tensor.matmul(sp0[:], ones_b[:, 0:128], ones_b[:],
                             start=True, stop=True)

        # ================= DMA issue (transfers run in background) =========
        # Three DMA rings in parallel so the startup critical path is
        # max(x chunk 0, q0/k0 bases) instead of their sum:
        #   sync:   x (first 4 tiles singly for early readiness, then groups),
        #           late v bases
        #   pool:   w_row + near-term weight bases
        #   scalar: light-unit q/k bases (ACT idle until first exp)
        w_row = const_p.tile([1, 4], FP32, tag="w_row")
        wall = [wall_p.tile([128, C], FP32, name=f"wall{j}", tag=f"wall{j}") for j in range(9)]
        xall = xall_p.tile([128, NT128 * C], FP32, tag="xall")
        xall3 = xall[:].rearrange("p (i c) -> p i c", c=C)
        x16 = x_d.ap().rearrange("(i p) c -> p i c", p=128)
        for tt in range(4):
            nc.sync.dma_start(xall3[:, tt:tt + 1, :], x16[:, tt:tt + 1, :])
        for grp in range(1, 4):
            nc.sync.dma_start(xall3[:, grp * 4:(grp + 1) * 4, :],
                              x16[:, grp * 4:(grp + 1) * 4, :])
        # j = 0..2 -> q(h0,h1,light), 3..5 -> k, 6..8 -> v; light packs
        # head2[0:64] + head3[0:64].
        nc.gpsimd.dma_start(w_row[:], w_d.ap().rearrange("a b -> b a"))
        nc.gpsimd.dma_start(wall[0][:], bq_d.ap()[0])
        nc.gpsimd.dma_start(wall[3][:], bk_d.ap()[0])
        nc.gpsimd.dma_start(wall[6][:], bv_d.ap()[0])
        nc.gpsimd.dma_start(wall[1][:], bq_d.ap()[1])
        nc.gpsimd.dma_start(wall[4][:], bk_d.ap()[1])
        nc.scalar.dma_start(wall[2][0:64, :], bq_d.ap()[2][0:64, :])
        nc.scalar.dma_start(wall[2][64:128, :], bq_d.ap()[3][0:64, :])
        nc.scalar.dma_start(wall[5][0:64, :], bk_d.ap()[2][0:64, :])
        nc.scalar.dma_start(wall[5][64:128, :], bk_d.ap()[3][0:64, :])
        nc.sync.dma_start(wall[7][:], bv_d.ap()[1])
        nc.sync.dma_start(wall[8][0:64, :], bv_d.ap()[2][0:64, :])
        nc.sync.dma_start(wall[8][64:128, :], bv_d.ap()[3][0:64, :])

        # ================= eff patterns (bf16 rank-1 matmuls) ============
        # effA[d, e] = sum_i w_i * (d < hs_i) * (e < emb_i)         (heads 0-3)
        # effB[d, e] = same for i in {1,3} with (d%64 < hs_i)       (packed light)
        HSL = (64, 32, 128, 64)
        EMB = (256, 256, 512, 512)
        wsc = [w_row[0:1, i:i + 1] for i in range(4)]
        effA = const_p.tile([128, C], FP32, tag="effA")
        effB = const_p.tile([128, C], FP32, tag="effB")
        for eff, cfgs, ext in ((effA, (0, 1, 2, 3), False), (effB, (1, 3), True)):
            p = ps.tile([128, 512], FP32, name="ps", tag="ps")
            for n, i in enumerate(cfgs):
                u = stage_p.tile([1, 128], BF16, name=f"u{i}{ext}", tag=f"u{i}{ext}", bufs=1)
                nc.vector.memset(u[:], 0.0)
                if ext:  # packed light: both 64-halves get the (d%64 < hs) pattern
                    nc.vector.memset(u[0:1, 0:min(HSL[i], 64)], 1.0)
                    nc.vector.memset(u[0:1, 64:64 + min(HSL[i], 64)], 1.0)
                else:
                    nc.vector.memset(u[0:1, 0:HSL[i]], 1.0)
                uw = stage_p.tile([1, 128], BF16, name=f"uw{i}{ext}", tag=f"uw{i}{ext}", bufs=1)
                nc.vector.tensor_scalar_mul(uw[:], u[:], wsc[i])
                vrow = stage_p.tile([1, C], BF16, name=f"vr{i}{ext}", tag=f"vr{i}{ext}", bufs=1)
                nc.vector.memset(vrow[:], 0.0)
                nc.vector.memset(vrow[0:1, 0:EMB[i]], 1.0)
                nc.tensor.matmul(p[:], uw[:], vrow[:],
                                 start=(n == 0), stop=(n == len(cfgs) - 1))
            nc.vector.tensor_copy(eff[:], p[:])

        # ================= persistent compute tensors ====================
        xt_all = xt_p.tile([128, NCC * T], BF16, tag="xt_all")
        xt = [xt_all[:, cc * T:(cc + 1) * T] for cc in range(NCC)]
        xt3 = xt_all[:].rearrange("p (a t) -> p a t", a=NCC)

        wtt = [wts_p.tile([128, 512], BF16, name=f"wtt{j}", tag=f"wtt{j}") for j in range(9)]
        wt = [[wtt[j][:, cc * 128:(cc + 1) * 128] for cc in range(NCC)]
              for j in range(9)]

        qt = [qk_p.tile([128, T], BF16, name=f"qt{h}", tag=f"qt{h}") for h in range(3)]
        kt = [qk_p.tile([128, T], BF16, name=f"kt{h}", tag=f"kt{h}") for h in range(3)]
        # vall[h]: 16 s-chunks of 132 cols. heavy: [v(0:128) | ones(128)];
        # light packs [v_l0(0:64) | ones(64) | v_l1(65:129) | ones(129)].
        # ones columns are set once via strided memsets and never rewritten.
        vall = [v_p.tile([128, NT128 * 132], BF16, name=f"vall{h}", tag=f"vall{h}")
                for h in range(3)]
        vall3 = [vall[h][:].rearrange("p (i c) -> p i c", c=132) for h in range(3)]
        nc.gpsimd.memset(vall3[0][:, :, 128:129], 1.0)
        nc.gpsimd.memset(vall3[1][:, :, 128:129], 1.0)
        nc.gpsimd.memset(vall3[2][:, :, 64:65], 1.0)
        nc.gpsimd.memset(vall3[2][:, :, 129:130], 1.0)

        pts = [pt_p.tile([128, (4 * tj + 4) * 512], BF16, name=f"pt{tj}", tag=f"pt{tj}")
               for tj in range(NT512)]

        # (proj unit, light half, v lo, v hi, out col)
        AU = [
            (0, None, 0, 129, 0),
            (1, None, 0, 129, 128),
            (2, 0, 0, 65, 256),
            (2, 1, 65, 130, 384),
        ]

        def kt_ap(au_i, i):
            pu, half = AU[au_i][0], AU[au_i][1]
            sl = slice(i * 128, (i + 1) * 128)
            if half is None:
                return kt[pu][:, sl]
            return kt[2][64 * half:64 * half + 64, sl]

        def qt_ap(au_i, tj):
            pu, half = AU[au_i][0], AU[au_i][1]
            sl = slice(tj * 512, (tj + 1) * 512)
            if half is None:
                return qt[pu][:, sl]
            return qt[2][64 * half:64 * half + 64, sl]

        # ================= work items (filler queue) =====================
        def xprep(tt):
            def fn():
                if tt < 4:
                    # startup tiles: fp32 transpose straight off the DMA, the
                    # PSUM->SBUF consolidation does the bf16 cast (Pool is
                    # busy issuing weight DMAs this early)
                    ptp = ps.tile([128, 512], FP32, name="ps", tag="ps")
                    for cc in range(NCC):
                        nc.tensor.transpose(
                            ptp[:, cc * 128:(cc + 1) * 128],
                            xall[:, tt * C + cc * 128:tt * C + (cc + 1) * 128],
                            ident_f[:])
                else:
                    # steady tiles: Pool casts to bf16, halving PE transpose
                    # cost and enabling the 2x DVE consolidation mode
                    xb = stage_p.tile([128, C], BF16, name="xb", tag="xb",
                                      bufs=3)
                    nc.gpsimd.tensor_copy(xb[:], xall[:, tt * C:(tt + 1) * C])
                    ptp = ps.tile([128, 512], BF16, name="ps", tag="ps")
                    for cc in range(NCC):
                        nc.tensor.transpose(
                            ptp[:, cc * 128:(cc + 1) * 128],
                            xb[:, cc * 128:(cc + 1) * 128], ident_b[:])
                nc.vector.tensor_copy(
                    xt3[:, :, tt * 128:(tt + 1) * 128],
                    ptp[:].rearrange("p (a t) -> p a t", a=NCC))
            return (700 if tt < 4 else 400, fn)

        def witem(j):
            def fn():
                w_bf = stage_p.tile([128, C], BF16, name="w_bf", tag="w_bf", bufs=2)
                nc.vector.tensor_mul(w_bf[:], wall[j][:],
                                     effB[:] if (j % 3) == 2 else effA[:])
                ptp = ps.tile([128, 512], BF16, name="ps", tag="ps")
                for cc in range(NCC):
                    nc.tensor.transpose(
                        ptp[:, cc * 128:(cc + 1) * 128],
                        w_bf[:, cc * 128:(cc + 1) * 128], ident_b[:])
                nc.vector.tensor_copy(wtt[j][:], ptp[:])
            return (400, fn)

        def qkitem(pu, tj, which):  # which: 0 = q, 1 = k
            def fn():
                j = 3 * which + pu
                dst = (qt if which == 0 else kt)[pu]
                p = ps.tile([128, 512], FP32, name="ps", tag="ps")
                for cc in range(NCC):
                    nc.tensor.matmul(
                        p[:], wt[j][cc][:], xt[cc][:, tj * 512:(tj + 1) * 512],
                        start=(cc == 0), stop=(cc == NCC - 1))
                nc.vector.tensor_copy(dst[:, tj * 512:(tj + 1) * 512], p[:])
            return (1000, fn)

        def vitem(pu, i):
            def fn():
                p = ps.tile([128, 512], FP32, name="ps", tag="ps")
                for cc in range(NCC):
                    nc.tensor.matmul(
                        p[:, 0:128], xt[cc][:, i * 128:(i + 1) * 128],
                        wt[6 + pu][cc][:],
                        start=(cc == 0), stop=(cc == NCC - 1))
                vt_i = vall[pu][:, i * 132:(i + 1) * 132]
                if pu < 2:
                    nc.vector.tensor_copy(vt_i[:, 0:128], p[:, 0:128])
                else:
                    nc.vector.tensor_copy(vt_i[:, 0:64], p[:, 0:64])
                    nc.vector.tensor_copy(vt_i[:, 65:129], p[:, 64:128])
            return (350, fn)

        # per-(unit, t-chunk) output stage: 4 normalized m-blocks gathered
        # into one SBUF tile, stored with a single strided DMA.
        out16 = out_d.ap().rearrange("(i p) c -> p i c", p=128)
        ob4_cur = {}

        def avitem(au_i, tj, m):
            pu, half, v_lo, v_hi, ocol = AU[au_i]
            w = v_hi - v_lo
            wm = w - 1
            ti = 4 * tj + m

            def fn():
                op = ps.tile([128, 512], FP32, name="ps", tag="ps")
                ptile = pts[tj]
                for i in range(ti + 1):
                    nc.tensor.matmul(
                        op[:, 0:w],
                        ptile[:, i * 512 + m * 128: i * 512 + (m + 1) * 128],
                        vall[pu][:, i * 132 + v_lo:i * 132 + v_hi],
                        start=(i == 0), stop=(i == ti))
                osb = osb_p.tile([128, 129], FP32, name="osb", tag="osb")
                nc.vector.tensor_copy(osb[:, 0:w], op[:, 0:w])
                if m == 0:
                    tag = "ob4h" if wm == 128 else "ob4l"
                    ob4_cur[au_i] = o_p.tile([128, 4 * wm], FP32,
                                             name=tag, tag=tag, bufs=3)
                ob4 = ob4_cur[au_i]
                nc.gpsimd.normalize_recip(
                    ob4[:, m * wm:(m + 1) * wm], osb[:, 0:wm], osb[:, wm:w])
                if m == 3:
                    nc.sync.dma_start(
                        out16[:, 4 * tj:4 * tj + 4, ocol:ocol + wm],
                        ob4[:].rearrange("p (m c) -> p m c", m=4))
            return ((ti + 1) * w * 0.46 + 250, fn)

        # ---- filler queue: emitted into PE gaps while ACT drains exp ----
        # Items are (deadline_chunk, cost_ns, fn). FIFO; deadlines force
        # emission before the consumer chunk starts (Tile resolves data deps
        # by emission order, so a producer must be emitted before its reader).
        fill = []

        def drain(budget):
            spent = 0.0
            while fill and spent < budget:
                _, cost, fn = fill.pop(0)
                fn()
                spent += cost

        def drain_due(ci):
            # emit every due item, preserving relative queue order (producer
            # deadlines are always <= their consumers', so this is dep-safe)
            due = [it for it in fill if it[0] <= ci]
            if due:
                fill[:] = [it for it in fill if it[0] > ci]
                for it in due:
                    it[2]()

        def drain_all():
            while fill:
                fill.pop(0)[2]()

        # ================= score chunks with budgeted filler ==============
        def sched_scores(au_i, tj):
            S = 4 * tj + 4
            ptile = pts[tj]
            g = 0
            while g < S:
                gw = min(SGRP, S - g)
                sp = sps.tile([128, SGRP * 512], FP32, name="sps", tag="sps")
                for k in range(gw):
                    i = g + k
                    nc.tensor.matmul(sp[:, k * 512:(k + 1) * 512],
                                     kt_ap(au_i, i), qt_ap(au_i, tj),
                                     start=True, stop=True)
                nc.scalar.activation(
                    ptile[:, g * 512:(g + gw) * 512],
                    sp[:, 0:gw * 512], AF.Exp, scale=SCALE)
                g += gw
                # ACT time of this group minus our own PE time
                drain(gw * 427 + 470 - gw * 240)
            # causal tri on the 4 diagonal 128-blocks, one fused op:
            # block r sits at 128-col index 16*tj + 5*r (stride 5)
            dst = ptile[:].rearrange("p (i q) -> p i q", q=128)[
                :, 16 * tj:16 * tj + 16:5, :]
            nc.vector.tensor_mul(dst, dst, tri3)

        # ================= main pipeline ==================================
        # HAM warm-up: keep PE streaming through startup DMA waits so the
        # clock gate reaches 8/8 before the first score chunk.
        def spin(n):
            for _ in range(n):
                sp0 = ps.tile([128, 512], FP32, name="ps", tag="ps")
                nc.tensor.matmul(sp0[:], ones_b[:, 0:128], ones_b[:],
                                 start=True, stop=True)

        # startup: weights q0/k0, x chunk 0, unit-0 qk proj for t-chunk 0
        spin(4)
        witem(0)[1]()
        witem(3)[1]()
        spin(4)
        for tt in range(4):
            xprep(tt)[1]()
        spin(2)
        qkitem(0, 0, 0)[1]()
        qkitem(0, 0, 1)[1]()

        # per-chunk filler: startup leftovers + next-unit projections + AV.
        # E(dl, item) attaches the mandatory-emission deadline chunk.
        def E(dl, item):
            return (dl, item[0], item[1])

        extras = {c: [] for c in range(16)}
        extras[0] += [E(1, witem(6)),
                      E(1, vitem(0, 0)), E(1, vitem(0, 1)),
                      E(1, vitem(0, 2)), E(1, vitem(0, 3)),
                      E(1, xprep(4)), E(1, xprep(5)),
                      E(1, xprep(6)), E(1, xprep(7)),
                      E(1, qkitem(0, 1, 0)), E(1, qkitem(0, 1, 1))]
        extras[1] += [E(2, xprep(8)), E(2, xprep(9)),
                      E(2, xprep(10)), E(2, xprep(11)),
                      E(2, qkitem(0, 2, 0)), E(2, qkitem(0, 2, 1)),
                      E(2, vitem(0, 4)), E(2, vitem(0, 5)),
                      E(2, vitem(0, 6)), E(2, vitem(0, 7)),
                      E(4, witem(1)), E(4, witem(4))]
        extras[2] += [E(3, xprep(12)), E(3, xprep(13)),
                      E(3, xprep(14)), E(3, xprep(15)),
                      E(3, qkitem(0, 3, 0)), E(3, qkitem(0, 3, 1)),
                      E(3, vitem(0, 8)), E(3, vitem(0, 9)),
                      E(3, vitem(0, 10)), E(3, vitem(0, 11)),
                      E(5, witem(7))]
        extras[3] += [E(4, vitem(0, 12)), E(4, vitem(0, 13)),
                      E(4, vitem(0, 14)), E(4, vitem(0, 15)),
                      E(8, witem(2)), E(8, witem(5)), E(8, witem(8))]
        # unit-0 windows carry proj of unit 1; unit-1 windows carry the
        # packed-light projections (units 2 and 3 share proj unit 2).
        for u in range(2):
            pu = u + 1
            base_c = 4 * pu
            for tj in range(NT512):
                c = 4 * u + tj
                extras[c] += [E(base_c + tj, qkitem(pu, tj, 0)),
                              E(base_c + tj, qkitem(pu, tj, 1))]
                extras[c] += [E(base_c + 1 + tj, vitem(pu, 4 * tj + r))
                              for r in range(4)]

        # unit 3 runs [2,3,1,0] so the tail AV after the last scores chunk
        # is the small tj=0 one (10 matmuls) instead of tj=3 (58).
        CH = ([(u, t) for u in range(3) for t in range(4)]
              + [(3, 2), (3, 3), (3, 1), (3, 0)])

        def av_deadline(ci, ptj):
            # AV of (pau, ptj) must be emitted before exp next rewrites
            # pts[ptj]; drain_due runs before sched_scores, so deadline may
            # equal that chunk's index.
            for c2 in range(ci, len(CH)):
                if CH[c2][1] == ptj:
                    return min(ci + 1, c2)
            return ci + 1

        for ci, (au_i, tj) in enumerate(CH):
            if ci > 0:
                pau, ptj = CH[ci - 1]
                dl = av_deadline(ci, ptj)
                for m in range(4):
                    fill.append(E(dl, avitem(pau, ptj, m)))
            fill.extend(extras[ci])
            drain_due(ci)
            sched_scores(au_i, tj)
        for m in range(4):
            avitem(3, 0, m)[1]()
        drain_all()


def _shard_inputs(x, weights, base_K, base_Q, base_V):
    in_maps = []
    for c in range(8):
        b = c // 2
        hsel = [0, 1, 4, 5] if c % 2 == 0 else [2, 3, 6, 7]
        in_maps.append({
            "x": np.ascontiguousarray(x[b]),
            "w": np.ascontiguousarray(weights.reshape(4, 1)),
            "bq": np.ascontiguousarray(base_Q[hsel]),
            "bk": np.ascontiguousarray(base_K[hsel]),
            "bv": np.ascontiguousarray(base_V[hsel]),
        })
    return in_maps


def _gather(results):
    out = np.zeros((4, T, 8 * HS), np.float32)
    for c in range(8):
        o = results[c]["out"]
        hsel = [0, 1, 4, 5] if c % 2 == 0 else [2, 3, 6, 7]
        for j, h in enumerate(hsel):
            out[c // 2][:, h * HS:(h + 1) * HS] = o[:, j * HS:(j + 1) * HS]
    return out


def get_nc():
    if "nc" not in _CACHE:
        _CACHE["nc"] = _build()
    return _CACHE["nc"]


def kernel(x, weights, base_K, base_Q, base_V):
    x = np.asarray(x, np.float32)
    weights = np.asarray(weights, np.float32)
    base_K = np.asarray(base_K, np.float32)
    base_Q = np.asarray(base_Q, np.float32)
    base_V = np.asarray(base_V, np.float32)
    nc = get_nc()
    in_maps = _shard_inputs(x, weights, base_K, base_Q, base_V)
    res = run_bass_kernel_spmd(nc, in_maps, core_ids=list(range(8)))
    return _gather(res.results)


# revision 19
# speedup vs baseline: 1.0550x; 1.0550x over previous
"""Trainium2 Bass kernel for nn_MixedHeadsV2 (mixed-head causal attention).

Full inputs in, full output out. Sharding: 8 cores = 4 batches x 2 head-groups.
Each core handles one batch and 4 of the 8 base heads: even cores heads
{0,1,4,5}, odd cores {2,3,6,7}. Heads 0-3 ("heavy") have effective head size
128; heads 4-7 ("light") have effective head size 64 (their mixed weight rows
64:128 are exactly zero), so the two light heads are packed into one 128-wide
tensor for projections and use 64-partition (K=64) score matmuls.

v2 engine plan (per core, Tile-scheduled, all instruction streams
software-pipelined so ACT/exp never starves and PE never idles):
  PE:   fp32 x-transposes, bf16 W-transposes, projections, scores, AV.
        AV of t-chunk tj is emitted during scores of chunk tj+1 (delay-1
        software pipeline); projections of the next unit and the x/W prep
        fill remaining PE slack via a cost-budgeted filler queue.
  ACT:  exp ONLY (scale folded), groups of 3 score tiles [128,1536] from a
        2-buf x 3-bank PSUM pool.
  DVE:  PSUM->SBUF glue (x consolidate+cast, qk casts, v copies, AV copies),
        fused 2-block causal tri masks.
  Pool: normalize_recip (out = av/denom, SBUF-side), vtile ones init,
        weight-DMA issue, affine_select consts.
  Sync: x loads + output stores.
"""
import sys

for p in ("/opt/trn_rl_repo",):
    if p not in sys.path:
        sys.path.append(p)

import numpy as np

import concourse.bass as bass
import concourse.tile as tile
from concourse import bacc, mybir
from concourse.bass_utils import run_bass_kernel_spmd

FP32 = mybir.dt.float32
BF16 = mybir.dt.bfloat16
AF = mybir.ActivationFunctionType
ALU = mybir.AluOpType

T = 2048
C = 512
HS = 128          # heavy head size (= padded head size)
NT128 = T // 128  # 16
NT512 = T // 512  # 4
NCC = C // 128    # 4
SCALE = float(1.0 / np.sqrt(128.0))
SGRP = 3          # score tiles (512 wide) per exp group / PSUM banks per buf

_CACHE = {}


def _build():
    nc = bacc.Bacc("TRN2", target_bir_lowering=False, debug=False, num_devices=8)
    x_d = nc.dram_tensor("x", [T, C], FP32, kind="ExternalInput")
    w_d = nc.dram_tensor("w", [4, 1], FP32, kind="ExternalInput")
    bq_d = nc.dram_tensor("bq", [4, HS, C], FP32, kind="ExternalInput")
    bk_d = nc.dram_tensor("bk", [4, HS, C], FP32, kind="ExternalInput")
    bv_d = nc.dram_tensor("bv", [4, HS, C], FP32, kind="ExternalInput")
    out_d = nc.dram_tensor("out", [T, 4 * HS], FP32, kind="ExternalOutput")

    with tile.TileContext(nc) as tc:
        _emit(nc, tc, x_d, w_d, bq_d, bk_d, bv_d, out_d)
    nc.compile()
    return nc


def _emit(nc, tc, x_d, w_d, bq_d, bk_d, bv_d, out_d):
    from contextlib import ExitStack

    ctx = ExitStack()
    with ctx:
        # ---- persistent SBUF pools ----
        const_p = ctx.enter_context(tc.tile_pool(name="const", bufs=1))
        wall_p = ctx.enter_context(tc.tile_pool(name="wall", bufs=1))
        wts_p = ctx.enter_context(tc.tile_pool(name="wts", bufs=1))
        xall_p = ctx.enter_context(tc.tile_pool(name="xall", bufs=1))
        xt_p = ctx.enter_context(tc.tile_pool(name="xt", bufs=1))
        qk_p = ctx.enter_context(tc.tile_pool(name="qk", bufs=1))
        v_p = ctx.enter_context(tc.tile_pool(name="v", bufs=1))
        pt_p = ctx.enter_context(tc.tile_pool(name="pt", bufs=1))
        osb_p = ctx.enter_context(tc.tile_pool(name="osb", bufs=8))
        o_p = ctx.enter_context(tc.tile_pool(name="o", bufs=4))
        stage_p = ctx.enter_context(tc.tile_pool(name="stage", bufs=2))
        # ---- PSUM: 2 bufs x 3 banks for score groups + 2 x 1 bank shared
        sps = ctx.enter_context(tc.tile_pool(name="sps", bufs=2, space="PSUM"))
        ps = ctx.enter_context(tc.tile_pool(name="ps", bufs=2, space="PSUM"))

        # ================= constants (first: Pool/DVE free, no DMA deps) ===
        ones_b = const_p.tile([128, C], BF16, tag="ones_b")
        nc.vector.memset(ones_b[:], 1.0)
        ident_b = const_p.tile([128, 128], BF16, tag="ident_b")
        nc.gpsimd.affine_select(
            ident_b[:], ones_b[:, 0:128], pattern=[[1, 128]],
            compare_op=ALU.is_equal, fill=0.0, base=0, channel_multiplier=-1)
        ident_f = const_p.tile([128, 128], FP32, tag="ident_f")
        nc.vector.tensor_copy(ident_f[:], ident_b[:])
        # causal triangle x4: tri4[:, r*128+t] = (t >= s) for r = 0..3
        tri4 = const_p.tile([128, 512], BF16, tag="tri4")
        for r in range(4):
            nc.gpsimd.affine_select(
                tri4[:, r * 128:(r + 1) * 128], ones_b[:, 0:128],
                pattern=[[1, 128]], compare_op=ALU.is_ge, fill=0.0, base=0,
                channel_multiplier=-1)
        tri3 = tri4[:].rearrange("p (r q) -> p r q", r=4)
        # start the PE streaming immediately (HAM clock-gate warm-up)
        for _ in range(3):
            sp0 = ps.tile([128, 512], FP32, name="ps", tag="ps")
            nc.tensor.matmul(sp0[:], ones_b[:, 0:128], ones_b[:],
                             start=True, stop=True)

        # ================= DMA issue (transfers run in background) =========
        # Three DMA rings in parallel so the startup critical path is
        # max(x chunk 0, q0/k0 bases) instead of their sum:
        #   sync:   x (first 4 tiles singly for early readiness, then groups),
        #           late v bases
        #   pool:   w_row + near-term weight bases
        #   scalar: light-unit q/k bases (ACT idle until first exp)
        w_row = const_p.tile([1, 4], FP32, tag="w_row")
        wall = [wall_p.tile([128, C], FP32, name=f"wall{j}", tag=f"wall{j}") for j in range(9)]
        xall = xall_p.tile([128, NT128 * C], FP32, tag="xall")
        xall3 = xall[:].rearrange("p (i c) -> p i c", c=C)
        x16 = x_d.ap().rearrange("(i p) c -> p i c", p=128)
        for tt in range(4):
            nc.sync.dma_start(xall3[:, tt:tt + 1, :], x16[:, tt:tt + 1, :])
        for grp in range(1, 4):
            nc.sync.dma_start(xall3[:, grp * 4:(grp + 1) * 4, :],
                              x16[:, grp * 4:(grp + 1) * 4, :])
        # j = 0..2 -> q(h0,h1,light), 3..5 -> k, 6..8 -> v; light packs
        # head2[0:64] + head3[0:64].
        nc.gpsimd.dma_start(w_row[:], w_d.ap().rearrange("a b -> b a"))
        nc.gpsimd.dma_start(wall[0][:], bq_d.ap()[0])
        nc.gpsimd.dma_start(wall[3][:], bk_d.ap()[0])
        nc.gpsimd.dma_start(wall[6][:], bv_d.ap()[0])
        nc.gpsimd.dma_start(wall[1][:], bq_d.ap()[1])
        nc.gpsimd.dma_start(wall[4][:], bk_d.ap()[1])
        nc.scalar.dma_start(wall[2][0:64, :], bq_d.ap()[2][0:64, :])
        nc.scalar.dma_start(wall[2][64:128, :], bq_d.ap()[3][0:64, :])
        nc.scalar.dma_start(wall[5][0:64, :], bk_d.ap()[2][0:64, :])
        nc.scalar.dma_start(wall[5][64:128, :], bk_d.ap()[3][0:64, :])
        nc.sync.dma_start(wall[7][:], bv_d.ap()[1])
        nc.sync.dma_start(wall[8][0:64, :], bv_d.ap()[2][0:64, :])
        nc.sync.dma_start(wall[8][64:128, :], bv_d.ap()[3][0:64, :])

        # ================= eff patterns (bf16 rank-1 matmuls) ============
        # effA[d, e] = sum_i w_i * (d < hs_i) * (e < emb_i)         (heads 0-3)
        # effB[d, e] = same for i in {1,3} with (d%64 < hs_i)       (packed light)
        HSL = (64, 32, 128, 64)
        EMB = (256, 256, 512, 512)
        wsc = [w_row[0:1, i:i + 1] for i in range(4)]
        effA = const_p.tile([128, C], FP32, tag="effA")
        effB = const_p.tile([128, C], FP32, tag="effB")
        for eff, cfgs, ext in ((effA, (0, 1, 2, 3), False), (effB, (1, 3), True)):
            p = ps.tile([128, 512], FP32, name="ps", tag="ps")
            for n, i in enumerate(cfgs):
                u = stage_p.tile([1, 128], BF16, name=f"u{i}{ext}", tag=f"u{i}{ext}", bufs=1)
                nc.vector.memset(u[:], 0.0)
                if ext:  # packed light: both 64-halves get the (d%64 < hs) pattern
                    nc.vector.memset(u[0:1, 0:min(HSL[i], 64)], 1.0)
                    nc.vector.memset(u[0:1, 64:64 + min(HSL[i], 64)], 1.0)
                else:
                    nc.vector.memset(u[0:1, 0:HSL[i]], 1.0)
                uw = stage_p.tile([1, 128], BF16, name=f"uw{i}{ext}", tag=f"uw{i}{ext}", bufs=1)
                nc.vector.tensor_scalar_mul(uw[:], u[:], wsc[i])
                vrow = stage_p.tile([1, C], BF16, name=f"vr{i}{ext}", tag=f"vr{i}{ext}", bufs=1)
                nc.vector.memset(vrow[:], 0.0)
                nc.vector.memset(vrow[0:1, 0:EMB[i]], 1.0)
                nc.tensor.matmul(p[:], uw[:], vrow[:],
                                 start=(n == 0), stop=(n == len(cfgs) - 1))
            nc.vector.tensor_copy(eff[:], p[:])

        # ================= persistent compute tensors ====================
        xt_all = xt_p.tile([128, NCC * T], BF16, tag="xt_all")
        xt = [xt_all[:, cc * T:(cc + 1) * T] for cc in range(NCC)]
        xt3 = xt_all[:].rearrange("p (a t) -> p a t", a=NCC)

        wtt = [wts_p.tile([128, 512], BF16, name=f"wtt{j}", tag=f"wtt{j}") for j in range(9)]
        wt = [[wtt[j][:, cc * 128:(cc + 1) * 128] for cc in range(NCC)]
              for j in range(9)]

        qt = [qk_p.tile([128, T], BF16, name=f"qt{h}", tag=f"qt{h}") for h in range(3)]
        kt = [qk_p.tile([128, T], BF16, name=f"kt{h}", tag=f"kt{h}") for h in range(3)]
        # vall[h]: 16 s-chunks of 132 cols. heavy: [v(0:128) | ones(128)];
        # light packs [v_l0(0:64) | ones(64) | v_l1(65:129) | ones(129)].
        # ones columns are set once via strided memsets and never rewritten.
        vall = [v_p.tile([128, NT128 * 132], BF16, name=f"vall{h}", tag=f"vall{h}")
                for h in range(3)]
        vall3 = [vall[h][:].rearrange("p (i c) -> p i c", c=132) for h in range(3)]
        nc.gpsimd.memset(vall3[0][:, :, 128:129], 1.0)
        nc.gpsimd.memset(vall3[1][:, :, 128:129], 1.0)
        nc.gpsimd.memset(vall3[2][:, :, 64:65], 1.0)
        nc.gpsimd.memset(vall3[2][:, :, 129:130], 1.0)

        pts = [pt_p.tile([128, (4 * tj + 4) * 512], BF16, name=f"pt{tj}", tag=f"pt{tj}")
               for tj in range(NT512)]

        # (proj unit, light half, v lo, v hi, out col)
        AU = [
            (0, None, 0, 129, 0),
            (1, None, 0, 129, 128),
            (2, 0, 0, 65, 256),
            (2, 1, 65, 130, 384),
        ]

        def kt_ap(au_i, i):
            pu, half = AU[au_i][0], AU[au_i][1]
            sl = slice(i * 128, (i + 1) * 128)
            if half is None:
                return kt[pu][:, sl]
            return kt[2][64 * half:64 * half + 64, sl]

        def qt_ap(au_i, tj):
            pu, half = AU[au_i][0], AU[au_i][1]
            sl = slice(tj * 512, (tj + 1) * 512)
            if half is None:
                return qt[pu][:, sl]
            return qt[2][64 * half:64 * half + 64, sl]

        # ================= work items (filler queue) =====================
        def xprep(tt):
            def fn():
                # fp32 transpose straight off the DMA; the PSUM->SBUF
                # consolidation does the bf16 cast (keeps Pool off the PE
                # critical path)
                ptp = ps.tile([128, 512], FP32, name="ps", tag="ps")
                for cc in range(NCC):
                    nc.tensor.transpose(
                        ptp[:, cc * 128:(cc + 1) * 128],
                        xall[:, tt * C + cc * 128:tt * C + (cc + 1) * 128],
                        ident_f[:])
                nc.vector.tensor_copy(
                    xt3[:, :, tt * 128:(tt + 1) * 128],
                    ptp[:].rearrange("p (a t) -> p a t", a=NCC))
            return (700, fn)

        def witem(j):
            def fn():
                w_bf = stage_p.tile([128, C], BF16, name="w_bf", tag="w_bf", bufs=2)
                nc.vector.tensor_mul(w_bf[:], wall[j][:],
                                     effB[:] if (j % 3) == 2 else effA[:])
                ptp = ps.tile([128, 512], BF16, name="ps", tag="ps")
                for cc in range(NCC):
                    nc.tensor.transpose(
                        ptp[:, cc * 128:(cc + 1) * 128],
                        w_bf[:, cc * 128:(cc + 1) * 128], ident_b[:])
                nc.vector.tensor_copy(wtt[j][:], ptp[:])
            return (400, fn)

        def qkitem(pu, tj, which):  # which: 0 = q, 1 = k
            def fn():
                j = 3 * which + pu
                dst = (qt if which == 0 else kt)[pu]
                p = ps.tile([128, 512], FP32, name="ps", tag="ps")
                for cc in range(NCC):
                    nc.tensor.matmul(
                        p[:], wt[j][cc][:], xt[cc][:, tj * 512:(tj + 1) * 512],
                        start=(cc == 0), stop=(cc == NCC - 1))
                nc.vector.tensor_copy(dst[:, tj * 512:(tj + 1) * 512], p[:])
            return (1000, fn)

        def vitem(pu, i):
            def fn():
                p = ps.tile([128, 512], FP32, name="ps", tag="ps")
                for cc in range(NCC):
                    nc.tensor.matmul(
                        p[:, 0:128], xt[cc][:, i * 128:(i + 1) * 128],
                        wt[6 + pu][cc][:],
                        start=(cc == 0), stop=(cc == NCC - 1))
                vt_i = vall[pu][:, i * 132:(i + 1) * 132]
                if pu < 2:
                    nc.vector.tensor_copy(vt_i[:, 0:128], p[:, 0:128])
                else:
                    nc.vector.tensor_copy(vt_i[:, 0:64], p[:, 0:64])
                    nc.vector.tensor_copy(vt_i[:, 65:129], p[:, 64:128])
            return (350, fn)

        # per-(unit, t-chunk) output stage: 4 normalized m-blocks gathered
        # into one SBUF tile, stored with a single strided DMA.
        out16 = out_d.ap().rearrange("(i p) c -> p i c", p=128)
        ob4_cur = {}

        def avitem(au_i, tj, m):
            pu, half, v_lo, v_hi, ocol = AU[au_i]
            w = v_hi - v_lo
            wm = w - 1
            ti = 4 * tj + m

            def fn():
                op = ps.tile([128, 512], FP32, name="ps", tag="ps")
                ptile = pts[tj]
                for i in range(ti + 1):
                    nc.tensor.matmul(
                        op[:, 0:w],
                        ptile[:, i * 512 + m * 128: i * 512 + (m + 1) * 128],
                        vall[pu][:, i * 132 + v_lo:i * 132 + v_hi],
                        start=(i == 0), stop=(i == ti))
                osb = osb_p.tile([128, 129], FP32, name="osb", tag="osb")
                nc.vector.tensor_copy(osb[:, 0:w], op[:, 0:w])
                if m == 0:
                    tag = "ob4h" if wm == 128 else "ob4l"
                    ob4_cur[au_i] = o_p.tile([128, 4 * wm], FP32,
                                             name=tag, tag=tag, bufs=3)
                ob4 = ob4_cur[au_i]
                nc.gpsimd.normalize_recip(
                    ob4[:, m * wm:(m + 1) * wm], osb[:, 0:wm], osb[:, wm:w])
                if m == 3:
                    nc.sync.dma_start(
                        out16[:, 4 * tj:4 * tj + 4, ocol:ocol + wm],
                        ob4[:].rearrange("p (m c) -> p m c", m=4))
            return ((ti + 1) * w * 0.46 + 250, fn)

        # ---- filler queue: emitted into PE gaps while ACT drains exp ----
        # Items are (deadline_chunk, cost_ns, fn). FIFO; deadlines force
        # emission before the consumer chunk starts (Tile resolves data deps
        # by emission order, so a producer must be emitted before its reader).
        fill = []

        def drain(budget):
            spent = 0.0
            while fill and spent < budget:
                _, cost, fn = fill.pop(0)
                fn()
                spent += cost

        def drain_due(ci):
            # emit every due item, preserving relative queue order (producer
            # deadlines are always <= their consumers', so this is dep-safe)
            due = [it for it in fill if it[0] <= ci]
            if due:
                fill[:] = [it for it in fill if it[0] > ci]
                for it in due:
                    it[2]()

        def drain_all():
            while fill:
                fill.pop(0)[2]()

        # ================= score chunks with budgeted filler ==============
        def sched_scores(au_i, tj):
            S = 4 * tj + 4
            ptile = pts[tj]
            g = 0
            while g < S:
                gw = min(SGRP, S - g)
                sp = sps.tile([128, SGRP * 512], FP32, name="sps", tag="sps")
                for k in range(gw):
                    i = g + k
                    nc.tensor.matmul(sp[:, k * 512:(k + 1) * 512],
                                     kt_ap(au_i, i), qt_ap(au_i, tj),
                                     start=True, stop=True)
                nc.scalar.activation(
                    ptile[:, g * 512:(g + gw) * 512],
                    sp[:, 0:gw * 512], AF.Exp, scale=SCALE)
                g += gw
                # ACT time of this group minus our own PE time
                drain(gw * 427 + 470 - gw * 240)
            # causal tri on the 4 diagonal 128-blocks, one fused op:
            # block r sits at 128-col index 16*tj + 5*r (stride 5)
            dst = ptile[:].rearrange("p (i q) -> p i q", q=128)[
                :, 16 * tj:16 * tj + 16:5, :]
            nc.vector.tensor_mul(dst, dst, tri3)

        # ================= main pipeline ==================================
        # HAM warm-up: keep PE streaming through startup DMA waits so the
        # clock gate reaches 8/8 before the first score chunk.
        def spin(n):
            for _ in range(n):
                sp0 = ps.tile([128, 512], FP32, name="ps", tag="ps")
                nc.tensor.matmul(sp0[:], ones_b[:, 0:128], ones_b[:],
                                 start=True, stop=True)

        # startup: weights q0/k0, x chunk 0, unit-0 qk proj for t-chunk 0
        spin(4)
        witem(0)[1]()
        witem(3)[1]()
        spin(4)
        for tt in range(4):
            xprep(tt)[1]()
        spin(2)
        qkitem(0, 0, 0)[1]()
        qkitem(0, 0, 1)[1]()

        # per-chunk filler: startup leftovers + next-unit projections + AV.
        # E(dl, item) attaches the mandatory-emission deadline chunk.
        def E(dl, item):
            return (dl, item[0], item[1])

        extras = {c: [] for c in range(16)}
        extras[0] += [E(1, witem(6)),
                      E(1, vitem(0, 0)), E(1, vitem(0, 1)),
                      E(1, vitem(0, 2)), E(1, vitem(0, 3)),
                      E(1, xprep(4)), E(1, xprep(5)),
                      E(1, xprep(6)), E(1, xprep(7)),
                      E(1, qkitem(0, 1, 0)), E(1, qkitem(0, 1, 1))]
        extras[1] += [E(2, xprep(8)), E(2, xprep(9)),
                      E(2, xprep(10)), E(2, xprep(11)),
                      E(2, qkitem(0, 2, 0)), E(2, qkitem(0, 2, 1)),
                      E(2, vitem(0, 4)), E(2, vitem(0, 5)),
                      E(2, vitem(0, 6)), E(2, vitem(0, 7)),
                      E(4, witem(1)), E(4, witem(4))]
        extras[2] += [E(3, xprep(12)), E(3, xprep(13)),
                      E(3, xprep(14)), E(3, xprep(15)),
                      E(3, qkitem(0, 3, 0)), E(3, qkitem(0, 3, 1)),
                      E(3, vitem(0, 8)), E(3, vitem(0, 9)),
                      E(3, vitem(0, 10)), E(3, vitem(0, 11)),
                      E(5, witem(7))]
        extras[3] += [E(4, vitem(0, 12)), E(4, vitem(0, 13)),
                      E(4, vitem(0, 14)), E(4, vitem(0, 15)),
                      E(8, witem(2)), E(8, witem(5)), E(8, witem(8))]
        # unit-0 windows carry proj of unit 1; unit-1 windows carry the
        # packed-light projections (units 2 and 3 share proj unit 2).
        for u in range(2):
            pu = u + 1
            base_c = 4 * pu
            for tj in range(NT512):
                c = 4 * u + tj
                extras[c] += [E(base_c + tj, qkitem(pu, tj, 0)),
                              E(base_c + tj, qkitem(pu, tj, 1))]
                extras[c] += [E(base_c + 1 + tj, vitem(pu, 4 * tj + r))
                              for r in range(4)]

        # unit 3 runs [2,3,1,0] so the tail AV after the last scores chunk
        # is the small tj=0 one (10 matmuls) instead of tj=3 (58).
        CH = ([(u, t) for u in range(3) for t in range(4)]
              + [(3, 2), (3, 3), (3, 1), (3, 0)])

        def av_deadline(ci, ptj):
            # AV of (pau, ptj) must be emitted before exp next rewrites
            # pts[ptj]; drain_due runs before sched_scores, so deadline may
            # equal that chunk's index.
            for c2 in range(ci, len(CH)):
                if CH[c2][1] == ptj:
                    return min(ci + 1, c2)
            return ci + 1

        for ci, (au_i, tj) in enumerate(CH):
            if ci > 0:
                pau, ptj = CH[ci - 1]
                dl = av_deadline(ci, ptj)
                for m in range(4):
                    fill.append(E(dl, avitem(pau, ptj, m)))
            fill.extend(extras[ci])
            drain_due(ci)
            sched_scores(au_i, tj)
        for m in range(4):
            avitem(3, 0, m)[1]()
        drain_all()


def _shard_inputs(x, weights, base_K, base_Q, base_V):
    in_maps = []
    for c in range(8):
        b = c // 2
        hsel = [0, 1, 4, 5] if c % 2 == 0 else [2, 3, 6, 7]
        in_maps.append({
            "x": np.ascontiguousarray(x[b]),
            "w": np.ascontiguousarray(weights.reshape(4, 1)),
            "bq": np.ascontiguousarray(base_Q[hsel]),
            "bk": np.ascontiguousarray(base_K[hsel]),
            "bv": np.ascontiguousarray(base_V[hsel]),
        })
    return in_maps


def _gather(results):
    out = np.zeros((4, T, 8 * HS), np.float32)
    for c in range(8):
        o = results[c]["out"]
        hsel = [0, 1, 4, 5] if c % 2 == 0 else [2, 3, 6, 7]
        for j, h in enumerate(hsel):
            out[c // 2][:, h * HS:(h + 1) * HS] = o[:, j * HS:(j + 1) * HS]
    return out


def get_nc():
    if "nc" not in _CACHE:
        _CACHE["nc"] = _build()
    return _CACHE["nc"]


def kernel(x, weights, base_K, base_Q, base_V):
    x = np.asarray(x, np.float32)
    weights = np.asarray(weights, np.float32)
    base_K = np.asarray(base_K, np.float32)
    base_Q = np.asarray(base_Q, np.float32)
    base_V = np.asarray(base_V, np.float32)
    nc = get_nc()
    in_maps = _shard_inputs(x, weights, base_K, base_Q, base_V)
    res = run_bass_kernel_spmd(nc, in_maps, core_ids=list(range(8)))
    return _gather(res.results)


# revision 28
# speedup vs baseline: 1.0727x; 1.0168x over previous
"""Trainium2 Bass kernel for nn_MixedHeadsV2 (mixed-head causal attention).

Full inputs in, full output out. Sharding: 8 cores = 4 batches x 2 head-groups.
Each core handles one batch and 4 of the 8 base heads: even cores heads
{0,1,4,5}, odd cores {2,3,6,7}. Heads 0-3 ("heavy") have effective head size
128; heads 4-7 ("light") have effective head size 64 (their mixed weight rows
64:128 are exactly zero), so the two light heads are packed into one 128-wide
tensor for projections and use 64-partition (K=64) score matmuls.

v2 engine plan (per core, Tile-scheduled, all instruction streams
software-pipelined so ACT/exp never starves and PE never idles):
  PE:   fp32 x-transposes, bf16 W-transposes, projections, scores, AV.
        AV of t-chunk tj is emitted during scores of chunk tj+1 (delay-1
        software pipeline); projections of the next unit and the x/W prep
        fill remaining PE slack via a cost-budgeted filler queue.
  ACT:  exp ONLY (scale folded), groups of 3 score tiles [128,1536] from a
        2-buf x 3-bank PSUM pool.
  DVE:  PSUM->SBUF glue (x consolidate+cast, qk casts, v copies, AV copies),
        fused 2-block causal tri masks.
  Pool: normalize_recip (out = av/denom, SBUF-side), vtile ones init,
        weight-DMA issue, affine_select consts.
  Sync: x loads + output stores.
"""
import sys

for p in ("/opt/trn_rl_repo",):
    if p not in sys.path:
        sys.path.append(p)

import numpy as np

import concourse.bass as bass
import concourse.tile as tile
from concourse import bacc, mybir
from concourse.bass_utils import run_bass_kernel_spmd

FP32 = mybir.dt.float32
BF16 = mybir.dt.bfloat16
AF = mybir.ActivationFunctionType
ALU = mybir.AluOpType

T = 2048
C = 512
HS = 128          # heavy head size (= padded head size)
NT128 = T // 128  # 16
NT512 = T // 512  # 4
NCC = C // 128    # 4
SCALE = float(1.0 / np.sqrt(128.0))
SGRP = 3          # score tiles (512 wide) per exp group / PSUM banks per buf

_CACHE = {}


def _build():
    nc = bacc.Bacc("TRN2", target_bir_lowering=False, debug=False, num_devices=8)
    x_d = nc.dram_tensor("x", [T, C], FP32, kind="ExternalInput")
    w_d = nc.dram_tensor("w", [4, 1], FP32, kind="ExternalInput")
    bq_d = nc.dram_tensor("bq", [4, HS, C], FP32, kind="ExternalInput")
    bk_d = nc.dram_tensor("bk", [4, HS, C], FP32, kind="ExternalInput")
    bv_d = nc.dram_tensor("bv", [4, HS, C], FP32, kind="ExternalInput")
    out_d = nc.dram_tensor("out", [T, 4 * HS], FP32, kind="ExternalOutput")

    with tile.TileContext(nc) as tc:
        _emit(nc, tc, x_d, w_d, bq_d, bk_d, bv_d, out_d)
    nc.compile()
    return nc


def _emit(nc, tc, x_d, w_d, bq_d, bk_d, bv_d, out_d):
    from contextlib import ExitStack

    ctx = ExitStack()
    with ctx:
        # ---- persistent SBUF pools ----
        const_p = ctx.enter_context(tc.tile_pool(name="const", bufs=1))
        wall_p = ctx.enter_context(tc.tile_pool(name="wall", bufs=1))
        wts_p = ctx.enter_context(tc.tile_pool(name="wts", bufs=1))
        xall_p = ctx.enter_context(tc.tile_pool(name="xall", bufs=1))
        xt_p = ctx.enter_context(tc.tile_pool(name="xt", bufs=1))
        qk_p = ctx.enter_context(tc.tile_pool(name="qk", bufs=1))
        v_p = ctx.enter_context(tc.tile_pool(name="v", bufs=1))
        pt_p = ctx.enter_context(tc.tile_pool(name="pt", bufs=1))
        osb_p = ctx.enter_context(tc.tile_pool(name="osb", bufs=8))
        o_p = ctx.enter_context(tc.tile_pool(name="o", bufs=4))
        stage_p = ctx.enter_context(tc.tile_pool(name="stage", bufs=2))
        # ---- PSUM: 2 bufs x 3 banks for score groups + 2 x 1 bank shared
        sps = ctx.enter_context(tc.tile_pool(name="sps", bufs=2, space="PSUM"))
        ps = ctx.enter_context(tc.tile_pool(name="ps", bufs=2, space="PSUM"))

        # ================= constants (first: Pool/DVE free, no DMA deps) ===
        ones_b = const_p.tile([128, C], BF16, tag="ones_b")
        nc.vector.memset(ones_b[:], 1.0)
        ident_b = const_p.tile([128, 128], BF16, tag="ident_b")
        nc.gpsimd.affine_select(
            ident_b[:], ones_b[:, 0:128], pattern=[[1, 128]],
            compare_op=ALU.is_equal, fill=0.0, base=0, channel_multiplier=-1)
        ident_f = const_p.tile([128, 128], FP32, tag="ident_f")
        nc.vector.tensor_copy(ident_f[:], ident_b[:])
        # causal triangle x4: tri4[:, r*128+t] = (t >= s) for r = 0..3
        tri4 = const_p.tile([128, 512], BF16, tag="tri4")
        for r in range(4):
            nc.gpsimd.affine_select(
                tri4[:, r * 128:(r + 1) * 128], ones_b[:, 0:128],
                pattern=[[1, 128]], compare_op=ALU.is_ge, fill=0.0, base=0,
                channel_multiplier=-1)
        tri3 = tri4[:].rearrange("p (r q) -> p r q", r=4)
        # start the PE streaming immediately (HAM clock-gate warm-up)
        for _ in range(3):
            sp0 = ps.tile([128, 512], FP32, name="ps", tag="ps")
            nc.tensor.matmul(sp0[:], ones_b[:, 0:128], ones_b[:],
                             start=True, stop=True)

        # ================= DMA issue (transfers run in background) =========
        # Three DMA rings in parallel so the startup critical path is
        # max(x chunk 0, q0/k0 bases) instead of their sum:
        #   sync:   x (first 4 tiles singly for early readiness, then groups),
        #           late v bases
        #   pool:   w_row + near-term weight bases
        #   scalar: light-unit q/k bases (ACT idle until first exp)
        w_row = const_p.tile([1, 4], FP32, tag="w_row")
        wall = [wall_p.tile([128, C], FP32, name=f"wall{j}", tag=f"wall{j}") for j in range(9)]
        xall = xall_p.tile([128, NT128 * C], FP32, tag="xall")
        xall3 = xall[:].rearrange("p (i c) -> p i c", c=C)
        x16 = x_d.ap().rearrange("(i p) c -> p i c", p=128)
        for tt in range(4):
            nc.sync.dma_start(xall3[:, tt:tt + 1, :], x16[:, tt:tt + 1, :])
        for grp in range(1, 4):
            nc.sync.dma_start(xall3[:, grp * 4:(grp + 1) * 4, :],
                              x16[:, grp * 4:(grp + 1) * 4, :])
        # j = 0..2 -> q(h0,h1,light), 3..5 -> k, 6..8 -> v; light packs
        # head2[0:64] + head3[0:64].
        nc.gpsimd.dma_start(w_row[:], w_d.ap().rearrange("a b -> b a"))
        nc.gpsimd.dma_start(wall[0][:], bq_d.ap()[0])
        nc.gpsimd.dma_start(wall[3][:], bk_d.ap()[0])
        nc.gpsimd.dma_start(wall[6][:], bv_d.ap()[0])
        nc.gpsimd.dma_start(wall[1][:], bq_d.ap()[1])
        nc.gpsimd.dma_start(wall[4][:], bk_d.ap()[1])
        nc.scalar.dma_start(wall[2][0:64, :], bq_d.ap()[2][0:64, :])
        nc.scalar.dma_start(wall[2][64:128, :], bq_d.ap()[3][0:64, :])
        nc.scalar.dma_start(wall[5][0:64, :], bk_d.ap()[2][0:64, :])
        nc.scalar.dma_start(wall[5][64:128, :], bk_d.ap()[3][0:64, :])
        nc.sync.dma_start(wall[7][:], bv_d.ap()[1])
        nc.sync.dma_start(wall[8][0:64, :], bv_d.ap()[2][0:64, :])
        nc.sync.dma_start(wall[8][64:128, :], bv_d.ap()[3][0:64, :])

        # ================= eff patterns (rank-1 K=1 matmuls) ===============
        # effA[d,e] = sum_i w_i * (d < hs_i) * (e < emb_i); light variant
        # effB over configs {1,3} uses the (d%64 < hs) pattern. All row
        # patterns live on partition 0 (engine partition-offset rule) in two
        # consolidated tiles: u4 blocks 0-3 heavy, 4-5 light; vrow blocks
        # emb=256 / emb=512.
        effA = const_p.tile([128, C], FP32, tag="effA")
        effB = const_p.tile([128, C], FP32, tag="effB")
        u4 = stage_p.tile([1, 6 * 128], BF16, name="u4", tag="u4", bufs=1)
        nc.vector.memset(u4[:], 0.0)
        nc.vector.memset(u4[0:1, 0:64], 1.0)        # cfg0 hs=64
        nc.vector.memset(u4[0:1, 128:160], 1.0)    # cfg1 hs=32
        nc.vector.memset(u4[0:1, 256:384], 1.0)    # cfg2 hs=128
        nc.vector.memset(u4[0:1, 384:448], 1.0)    # cfg3 hs=64
        nc.vector.memset(u4[0:1, 512:544], 1.0)    # light cfg1: d%64<32
        nc.vector.memset(u4[0:1, 576:608], 1.0)
        nc.vector.memset(u4[0:1, 640:768], 1.0)    # light cfg3: all
        vrow = stage_p.tile([1, 2 * C], BF16, name="vrow", tag="vrow", bufs=1)
        nc.vector.memset(vrow[:], 0.0)
        nc.vector.memset(vrow[0:1, 0:256], 1.0)    # emb=256 pattern
        nc.vector.memset(vrow[0:1, C:2 * C], 1.0)  # emb=512 pattern
        uw4 = stage_p.tile([1, 6 * 128], BF16, name="uw4", tag="uw4", bufs=1)
        for i in range(6):
            wi = (0, 1, 2, 3, 1, 3)[i]
            nc.vector.tensor_scalar_mul(
                uw4[0:1, i * 128:(i + 1) * 128],
                u4[0:1, i * 128:(i + 1) * 128], w_row[0:1, wi:wi + 1])
        EMBSEL = (0, 0, 1, 1, 0, 1)  # which vrow block per config
        pE = ps.tile([128, 512], FP32, name="ps", tag="ps")
        for n, i in enumerate((0, 1, 2, 3)):
            nc.tensor.matmul(pE[:], uw4[0:1, i * 128:(i + 1) * 128],
                             vrow[0:1, EMBSEL[i] * C:(EMBSEL[i] + 1) * C],
                             start=(n == 0), stop=(n == 3))
        nc.vector.tensor_copy(effA[:], pE[:])
        pE2 = ps.tile([128, 512], FP32, name="ps", tag="ps")
        for n, i in enumerate((4, 5)):
            nc.tensor.matmul(pE2[:], uw4[0:1, i * 128:(i + 1) * 128],
                             vrow[0:1, EMBSEL[i] * C:(EMBSEL[i] + 1) * C],
                             start=(n == 0), stop=(n == 1))
        nc.vector.tensor_copy(effB[:], pE2[:])

        # ================= persistent compute tensors ====================
        xt_all = xt_p.tile([128, NCC * T], BF16, tag="xt_all")
        xt = [xt_all[:, cc * T:(cc + 1) * T] for cc in range(NCC)]
        xt3 = xt_all[:].rearrange("p (a t) -> p a t", a=NCC)

        wtt = [wts_p.tile([128, 512], BF16, name=f"wtt{j}", tag=f"wtt{j}") for j in range(9)]
        wt = [[wtt[j][:, cc * 128:(cc + 1) * 128] for cc in range(NCC)]
              for j in range(9)]

        qt = [qk_p.tile([128, T], BF16, name=f"qt{h}", tag=f"qt{h}") for h in range(3)]
        kt = [qk_p.tile([128, T], BF16, name=f"kt{h}", tag=f"kt{h}") for h in range(3)]
        # vall[h]: 16 s-chunks of 132 cols. heavy: [v(0:128) | ones(128)];
        # light packs [v_l0(0:64) | ones(64) | v_l1(65:129) | ones(129)].
        # ones columns are set once via strided memsets and never rewritten.
        vall = [v_p.tile([128, NT128 * 132], BF16, name=f"vall{h}", tag=f"vall{h}")
                for h in range(3)]
        vall3 = [vall[h][:].rearrange("p (i c) -> p i c", c=132) for h in range(3)]
        nc.gpsimd.memset(vall3[0][:, :, 128:129], 1.0)
        nc.gpsimd.memset(vall3[1][:, :, 128:129], 1.0)
        nc.gpsimd.memset(vall3[2][:, :, 64:65], 1.0)
        nc.gpsimd.memset(vall3[2][:, :, 129:130], 1.0)

        pts = [pt_p.tile([128, (4 * tj + 4) * 512], BF16, name=f"pt{tj}", tag=f"pt{tj}")
               for tj in range(NT512)]

        # (proj unit, light half, v lo, v hi, out col)
        AU = [
            (0, None, 0, 129, 0),
            (1, None, 0, 129, 128),
            (2, 0, 0, 65, 256),
            (2, 1, 65, 130, 384),
        ]

        def kt_ap(au_i, i):
            pu, half = AU[au_i][0], AU[au_i][1]
            sl = slice(i * 128, (i + 1) * 128)
            if half is None:
                return kt[pu][:, sl]
            return kt[2][64 * half:64 * half + 64, sl]

        def qt_ap(au_i, tj):
            pu, half = AU[au_i][0], AU[au_i][1]
            sl = slice(tj * 512, (tj + 1) * 512)
            if half is None:
                return qt[pu][:, sl]
            return qt[2][64 * half:64 * half + 64, sl]

        # ================= work items (filler queue) =====================
        def xprep(tt):
            def fn():
                # fp32 transpose straight off the DMA; the PSUM->SBUF
                # consolidation does the bf16 cast. During startup ACT is
                # idle, so the first four consolidations run there instead
                # of on the (startup-critical) DVE.
                ptp = ps.tile([128, 512], FP32, name="ps", tag="ps")
                for cc in range(NCC):
                    nc.tensor.transpose(
                        ptp[:, cc * 128:(cc + 1) * 128],
                        xall[:, tt * C + cc * 128:tt * C + (cc + 1) * 128],
                        ident_f[:])
                nc.vector.tensor_copy(
                    xt3[:, :, tt * 128:(tt + 1) * 128],
                    ptp[:].rearrange("p (a t) -> p a t", a=NCC))
            return (700, fn)

        def witem(j):
            def fn():
                w_bf = stage_p.tile([128, C], BF16, name="w_bf", tag="w_bf", bufs=2)
                nc.vector.tensor_mul(w_bf[:], wall[j][:],
                                     effB[:] if (j % 3) == 2 else effA[:])
                ptp = ps.tile([128, 512], BF16, name="ps", tag="ps")
                for cc in range(NCC):
                    nc.tensor.transpose(
                        ptp[:, cc * 128:(cc + 1) * 128],
                        w_bf[:, cc * 128:(cc + 1) * 128], ident_b[:])
                cp = nc.scalar.copy if j in (0, 3) else nc.vector.tensor_copy
                cp(wtt[j][:], ptp[:])
            return (400, fn)

        def qkitem(pu, tj, which):  # which: 0 = q, 1 = k
            def fn():
                j = 3 * which + pu
                dst = (qt if which == 0 else kt)[pu]
                p = ps.tile([128, 512], FP32, name="ps", tag="ps")
                for cc in range(NCC):
                    nc.tensor.matmul(
                        p[:], wt[j][cc][:], xt[cc][:, tj * 512:(tj + 1) * 512],
                        start=(cc == 0), stop=(cc == NCC - 1))
                cp = (nc.scalar.copy if (pu == 0 and tj == 0)
                      else nc.vector.tensor_copy)
                cp(dst[:, tj * 512:(tj + 1) * 512], p[:])
            return (1000, fn)

        def vitem(pu, i):
            def fn():
                p = ps.tile([128, 512], FP32, name="ps", tag="ps")
                for cc in range(NCC):
                    nc.tensor.matmul(
                        p[:, 0:128], xt[cc][:, i * 128:(i + 1) * 128],
                        wt[6 + pu][cc][:],
                        start=(cc == 0), stop=(cc == NCC - 1))
                vt_i = vall[pu][:, i * 132:(i + 1) * 132]
                if pu < 2:
                    nc.vector.tensor_copy(vt_i[:, 0:128], p[:, 0:128])
                else:
                    nc.vector.tensor_copy(vt_i[:, 0:64], p[:, 0:64])
                    nc.vector.tensor_copy(vt_i[:, 65:129], p[:, 64:128])
            return (350, fn)

        # per-(unit, t-chunk) output stage: 4 normalized m-blocks gathered
        # into one SBUF tile, stored with a single strided DMA.
        out16 = out_d.ap().rearrange("(i p) c -> p i c", p=128)
        ob4_cur = {}

        def avitem(au_i, tj, m):
            pu, half, v_lo, v_hi, ocol = AU[au_i]
            w = v_hi - v_lo
            wm = w - 1
            ti = 4 * tj + m

            def fn():
                op = ps.tile([128, 512], FP32, name="ps", tag="ps")
                ptile = pts[tj]
                for i in range(ti + 1):
                    nc.tensor.matmul(
                        op[:, 0:w],
                        ptile[:, i * 512 + m * 128: i * 512 + (m + 1) * 128],
                        vall[pu][:, i * 132 + v_lo:i * 132 + v_hi],
                        start=(i == 0), stop=(i == ti))
                osb = osb_p.tile([128, 129], FP32, name="osb", tag="osb")
                nc.vector.tensor_copy(osb[:, 0:w], op[:, 0:w])
                if m == 0:
                    tag = "ob4h" if wm == 128 else "ob4l"
                    ob4_cur[au_i] = o_p.tile([128, 4 * wm], FP32,
                                             name=tag, tag=tag, bufs=3)
                ob4 = ob4_cur[au_i]
                nc.gpsimd.normalize_recip(
                    ob4[:, m * wm:(m + 1) * wm], osb[:, 0:wm], osb[:, wm:w])
                if m == 3:
                    nc.sync.dma_start(
                        out16[:, 4 * tj:4 * tj + 4, ocol:ocol + wm],
                        ob4[:].rearrange("p (m c) -> p m c", m=4))
            return ((ti + 1) * w * 0.46 + 250, fn)

        # ---- filler queue: emitted into PE gaps while ACT drains exp ----
        # Items are (deadline_chunk, cost_ns, fn). FIFO; deadlines force
        # emission before the consumer chunk starts (Tile resolves data deps
        # by emission order, so a producer must be emitted before its reader).
        fill = []

        def drain(budget):
            spent = 0.0
            while fill and spent < budget:
                _, cost, fn = fill.pop(0)
                fn()
                spent += cost

        def drain_due(ci):
            # emit every due item, preserving relative queue order (producer
            # deadlines are always <= their consumers', so this is dep-safe)
            due = [it for it in fill if it[0] <= ci]
            if due:
                fill[:] = [it for it in fill if it[0] > ci]
                for it in due:
                    it[2]()

        def drain_all():
            while fill:
                fill.pop(0)[2]()

        # ================= score chunks with budgeted filler ==============
        def sched_scores(au_i, tj, final=False):
            S = 4 * tj + 4
            ptile = pts[tj]
            g = 0
            done_av = 0
            while g < S:
                gw = min(SGRP, S - g)
                sp = sps.tile([128, SGRP * 512], FP32, name="sps", tag="sps")
                for k in range(gw):
                    i = g + k
                    nc.tensor.matmul(sp[:, k * 512:(k + 1) * 512],
                                     kt_ap(au_i, i), qt_ap(au_i, tj),
                                     start=True, stop=True)
                nc.scalar.activation(
                    ptile[:, g * 512:(g + gw) * 512],
                    sp[:, 0:gw * 512], AF.Exp, scale=SCALE)
                g += gw
                if final:
                    # last chunk (tj == 0): mask + AV per diagonal block as
                    # soon as its exp lands, so the tail is a single m-block
                    while done_av < min(g, 4):
                        r = done_av
                        blk = ptile[:, r * 512 + r * 128:
                                    r * 512 + (r + 1) * 128]
                        nc.vector.tensor_mul(
                            blk, blk, tri4[:, r * 128:(r + 1) * 128])
                        avitem(au_i, tj, r)[1]()
                        done_av += 1
                else:
                    # ACT time of this group minus our own PE time
                    drain(gw * 427 + 470 - gw * 240)
            if not final:
                # causal tri on the 4 diagonal 128-blocks, one fused op:
                # block r sits at 128-col index 16*tj + 5*r (stride 5)
                dst = ptile[:].rearrange("p (i q) -> p i q", q=128)[
                    :, 16 * tj:16 * tj + 16:5, :]
                nc.vector.tensor_mul(dst, dst, tri3)

        # ================= main pipeline ==================================
        # HAM warm-up: keep PE streaming through startup DMA waits so the
        # clock gate reaches 8/8 before the first score chunk.
        def spin(n):
            for _ in range(n):
                sp0 = ps.tile([128, 512], FP32, name="ps", tag="ps")
                nc.tensor.matmul(sp0[:], ones_b[:, 0:128], ones_b[:],
                                 start=True, stop=True)

        # startup: weights q0/k0, x chunk 0, unit-0 qk proj for t-chunk 0
        spin(4)
        witem(0)[1]()
        witem(3)[1]()
        spin(4)
        for tt in range(4):
            xprep(tt)[1]()
        spin(2)
        qkitem(0, 0, 0)[1]()
        qkitem(0, 0, 1)[1]()

        # per-chunk filler: startup leftovers + next-unit projections + AV.
        # E(dl, item) attaches the mandatory-emission deadline chunk.
        def E(dl, item):
            return (dl, item[0], item[1])

        extras = {c: [] for c in range(16)}
        extras[0] += [E(1, witem(6)),
                      E(1, vitem(0, 0)), E(1, vitem(0, 1)),
                      E(1, vitem(0, 2)), E(1, vitem(0, 3)),
                      E(1, xprep(4)), E(1, xprep(5)),
                      E(1, xprep(6)), E(1, xprep(7)),
                      E(1, qkitem(0, 1, 0)), E(1, qkitem(0, 1, 1))]
        extras[1] += [E(2, xprep(8)), E(2, xprep(9)),
                      E(2, xprep(10)), E(2, xprep(11)),
                      E(2, qkitem(0, 2, 0)), E(2, qkitem(0, 2, 1)),
                      E(2, vitem(0, 4)), E(2, vitem(0, 5)),
                      E(2, vitem(0, 6)), E(2, vitem(0, 7)),
                      E(4, witem(1)), E(4, witem(4))]
        extras[2] += [E(3, xprep(12)), E(3, xprep(13)),
                      E(3, xprep(14)), E(3, xprep(15)),
                      E(3, qkitem(0, 3, 0)), E(3, qkitem(0, 3, 1)),
                      E(3, vitem(0, 8)), E(3, vitem(0, 9)),
                      E(3, vitem(0, 10)), E(3, vitem(0, 11)),
                      E(5, witem(7))]
        extras[3] += [E(4, vitem(0, 12)), E(4, vitem(0, 13)),
                      E(4, vitem(0, 14)), E(4, vitem(0, 15)),
                      E(8, witem(2)), E(8, witem(5)), E(8, witem(8))]
        # unit-0 windows carry proj of unit 1; unit-1 windows carry the
        # packed-light projections (units 2 and 3 share proj unit 2).
        for u in range(2):
            pu = u + 1
            base_c = 4 * pu
            for tj in range(NT512):
                c = 4 * u + tj
                extras[c] += [E(base_c + tj, qkitem(pu, tj, 0)),
                              E(base_c + tj, qkitem(pu, tj, 1))]
                extras[c] += [E(base_c + 1 + tj, vitem(pu, 4 * tj + r))
                              for r in range(4)]

        # unit 3 runs [2,3,1,0] so the tail AV after the last scores chunk
        # is the small tj=0 one (10 matmuls) instead of tj=3 (58).
        CH = ([(u, t) for u in range(3) for t in range(4)]
              + [(3, 2), (3, 3), (3, 1), (3, 0)])

        def av_deadline(ci, ptj):
            # AV of (pau, ptj) must be emitted before exp next rewrites
            # pts[ptj]; drain_due runs before sched_scores, so deadline may
            # equal that chunk's index.
            for c2 in range(ci, len(CH)):
                if CH[c2][1] == ptj:
                    return min(ci + 1, c2)
            return ci + 1

        for ci, (au_i, tj) in enumerate(CH):
            last = ci == len(CH) - 1
            if ci > 0:
                pau, ptj = CH[ci - 1]
                dl = ci if last else av_deadline(ci, ptj)
                for m in range(4):
                    fill.append(E(dl, avitem(pau, ptj, m)))
            fill.extend(extras[ci])
            drain_due(ci)
            sched_scores(au_i, tj, final=last)
        drain_all()


def _shard_inputs(x, weights, base_K, base_Q, base_V):
    in_maps = []
    for c in range(8):
        b = c // 2
        hsel = [0, 1, 4, 5] if c % 2 == 0 else [2, 3, 6, 7]
        in_maps.append({
            "x": np.ascontiguousarray(x[b]),
            "w": np.ascontiguousarray(weights.reshape(4, 1)),
            "bq": np.ascontiguousarray(base_Q[hsel]),
            "bk": np.ascontiguousarray(base_K[hsel]),
            "bv": np.ascontiguousarray(base_V[hsel]),
        })
    return in_maps


def _gather(results):
    out = np.zeros((4, T, 8 * HS), np.float32)
    for c in range(8):
        o = results[c]["out"]
        hsel = [0, 1, 4, 5] if c % 2 == 0 else [2, 3, 6, 7]
        for j, h in enumerate(hsel):
            out[c // 2][:, h * HS:(h + 1) * HS] = o[:, j * HS:(j + 1) * HS]
    return out


def get_nc():
    if "nc" not in _CACHE:
        _CACHE["nc"] = _build()
    return _CACHE["nc"]


def kernel(x, weights, base_K, base_Q, base_V):
    x = np.asarray(x, np.float32)
    weights = np.asarray(weights, np.float32)
    base_K = np.asarray(base_K, np.float32)
    base_Q = np.asarray(base_Q, np.float32)
    base_V = np.asarray(base_V, np.float32)
    nc = get_nc()
    in_maps = _shard_inputs(x, weights, base_K, base_Q, base_V)
    res = run_bass_kernel_spmd(nc, in_maps, core_ids=list(range(8)))
    return _gather(res.results)


# revision 34
# speedup vs baseline: 1.0771x; 1.0040x over previous
"""Trainium2 Bass kernel for nn_MixedHeadsV2 (mixed-head causal attention).

Full inputs in, full output out. Sharding: 8 cores = 4 batches x 2 head-groups.
Each core handles one batch and 4 of the 8 base heads: even cores heads
{0,1,4,5}, odd cores {2,3,6,7}. Heads 0-3 ("heavy") have effective head size
128; heads 4-7 ("light") have effective head size 64 (their mixed weight rows
64:128 are exactly zero), so the two light heads are packed into one 128-wide
tensor for projections and use 64-partition (K=64) score matmuls.

v2 engine plan (per core, Tile-scheduled, all instruction streams
software-pipelined so ACT/exp never starves and PE never idles):
  PE:   fp32 x-transposes, bf16 W-transposes, projections, scores, AV.
        AV of t-chunk tj is emitted during scores of chunk tj+1 (delay-1
        software pipeline); projections of the next unit and the x/W prep
        fill remaining PE slack via a cost-budgeted filler queue.
  ACT:  exp ONLY (scale folded), groups of 3 score tiles [128,1536] from a
        2-buf x 3-bank PSUM pool.
  DVE:  PSUM->SBUF glue (x consolidate+cast, qk casts, v copies, AV copies),
        fused 2-block causal tri masks.
  Pool: normalize_recip (out = av/denom, SBUF-side), vtile ones init,
        weight-DMA issue, affine_select consts.
  Sync: x loads + output stores.
"""
import sys

for p in ("/opt/trn_rl_repo",):
    if p not in sys.path:
        sys.path.append(p)

import numpy as np

import concourse.bass as bass
import concourse.tile as tile
from concourse import bacc, mybir
from concourse.bass_utils import run_bass_kernel_spmd

FP32 = mybir.dt.float32
BF16 = mybir.dt.bfloat16
AF = mybir.ActivationFunctionType
ALU = mybir.AluOpType

T = 2048
C = 512
HS = 128          # heavy head size (= padded head size)
NT128 = T // 128  # 16
NT512 = T // 512  # 4
NCC = C // 128    # 4
SCALE = float(1.0 / np.sqrt(128.0))
SGRP = 2          # score tiles (512 wide) per exp group / PSUM banks per buf

_CACHE = {}


def _build():
    nc = bacc.Bacc("TRN2", target_bir_lowering=False, debug=False, num_devices=8)
    x_d = nc.dram_tensor("x", [T, C], FP32, kind="ExternalInput")
    w_d = nc.dram_tensor("w", [4, 1], FP32, kind="ExternalInput")
    bq_d = nc.dram_tensor("bq", [4, HS, C], FP32, kind="ExternalInput")
    bk_d = nc.dram_tensor("bk", [4, HS, C], FP32, kind="ExternalInput")
    bv_d = nc.dram_tensor("bv", [4, HS, C], FP32, kind="ExternalInput")
    out_d = nc.dram_tensor("out", [T, 4 * HS], FP32, kind="ExternalOutput")

    with tile.TileContext(nc) as tc:
        _emit(nc, tc, x_d, w_d, bq_d, bk_d, bv_d, out_d)
    nc.compile()
    return nc


def _emit(nc, tc, x_d, w_d, bq_d, bk_d, bv_d, out_d):
    from contextlib import ExitStack

    ctx = ExitStack()
    with ctx:
        # ---- persistent SBUF pools ----
        const_p = ctx.enter_context(tc.tile_pool(name="const", bufs=1))
        wall_p = ctx.enter_context(tc.tile_pool(name="wall", bufs=1))
        wts_p = ctx.enter_context(tc.tile_pool(name="wts", bufs=1))
        xall_p = ctx.enter_context(tc.tile_pool(name="xall", bufs=1))
        xt_p = ctx.enter_context(tc.tile_pool(name="xt", bufs=1))
        qk_p = ctx.enter_context(tc.tile_pool(name="qk", bufs=1))
        v_p = ctx.enter_context(tc.tile_pool(name="v", bufs=1))
        pt_p = ctx.enter_context(tc.tile_pool(name="pt", bufs=1))
        osb_p = ctx.enter_context(tc.tile_pool(name="osb", bufs=8))
        o_p = ctx.enter_context(tc.tile_pool(name="o", bufs=4))
        stage_p = ctx.enter_context(tc.tile_pool(name="stage", bufs=2))
        # ---- PSUM: 2 bufs x 2 banks for score groups, 2 x AV-chain bufs,
        # 2 x 1 bank for projections/transposes (separate pools so an AV
        # chain never waits on a projection's PSUM->SBUF cast)
        sps = ctx.enter_context(tc.tile_pool(name="sps", bufs=2, space="PSUM"))
        pav = ctx.enter_context(tc.tile_pool(name="pav", bufs=2, space="PSUM"))
        ps = ctx.enter_context(tc.tile_pool(name="ps", bufs=2, space="PSUM"))

        # ================= constants (first: Pool/DVE free, no DMA deps) ===
        ones_b = const_p.tile([128, C], BF16, tag="ones_b")
        nc.vector.memset(ones_b[:], 1.0)
        ident_b = const_p.tile([128, 128], BF16, tag="ident_b")
        nc.gpsimd.affine_select(
            ident_b[:], ones_b[:, 0:128], pattern=[[1, 128]],
            compare_op=ALU.is_equal, fill=0.0, base=0, channel_multiplier=-1)
        ident_f = const_p.tile([128, 128], FP32, tag="ident_f")
        nc.vector.tensor_copy(ident_f[:], ident_b[:])
        # causal triangle x4: tri4[:, r*128+t] = (t >= s) for r = 0..3
        tri4 = const_p.tile([128, 512], BF16, tag="tri4")
        for r in range(4):
            nc.gpsimd.affine_select(
                tri4[:, r * 128:(r + 1) * 128], ones_b[:, 0:128],
                pattern=[[1, 128]], compare_op=ALU.is_ge, fill=0.0, base=0,
                channel_multiplier=-1)
        tri3 = tri4[:].rearrange("p (r q) -> p r q", r=4)
        # start the PE streaming immediately (HAM clock-gate warm-up)
        for _ in range(3):
            sp0 = ps.tile([128, 512], FP32, name="ps", tag="ps")
            nc.tensor.matmul(sp0[:], ones_b[:, 0:128], ones_b[:],
                             start=True, stop=True)

        # ================= DMA issue (transfers run in background) =========
        # Three DMA rings in parallel so the startup critical path is
        # max(x chunk 0, q0/k0 bases) instead of their sum:
        #   sync:   x (first 4 tiles singly for early readiness, then groups),
        #           late v bases
        #   pool:   w_row + near-term weight bases
        #   scalar: light-unit q/k bases (ACT idle until first exp)
        w_row = const_p.tile([1, 4], FP32, tag="w_row")
        wall = [wall_p.tile([128, C], FP32, name=f"wall{j}", tag=f"wall{j}") for j in range(9)]
        xall = xall_p.tile([128, NT128 * C], FP32, tag="xall")
        xall3 = xall[:].rearrange("p (i c) -> p i c", c=C)
        x16 = x_d.ap().rearrange("(i p) c -> p i c", p=128)
        for tt in range(4):
            nc.sync.dma_start(xall3[:, tt:tt + 1, :], x16[:, tt:tt + 1, :])
        for grp in range(1, 4):
            nc.sync.dma_start(xall3[:, grp * 4:(grp + 1) * 4, :],
                              x16[:, grp * 4:(grp + 1) * 4, :])
        # j = 0..2 -> q(h0,h1,light), 3..5 -> k, 6..8 -> v; light packs
        # head2[0:64] + head3[0:64].
        nc.gpsimd.dma_start(w_row[:], w_d.ap().rearrange("a b -> b a"))
        nc.gpsimd.dma_start(wall[0][:], bq_d.ap()[0])
        nc.gpsimd.dma_start(wall[3][:], bk_d.ap()[0])
        nc.gpsimd.dma_start(wall[6][:], bv_d.ap()[0])
        nc.gpsimd.dma_start(wall[1][:], bq_d.ap()[1])
        nc.gpsimd.dma_start(wall[4][:], bk_d.ap()[1])
        nc.scalar.dma_start(wall[2][0:64, :], bq_d.ap()[2][0:64, :])
        nc.scalar.dma_start(wall[2][64:128, :], bq_d.ap()[3][0:64, :])
        nc.scalar.dma_start(wall[5][0:64, :], bk_d.ap()[2][0:64, :])
        nc.scalar.dma_start(wall[5][64:128, :], bk_d.ap()[3][0:64, :])
        nc.sync.dma_start(wall[7][:], bv_d.ap()[1])
        nc.sync.dma_start(wall[8][0:64, :], bv_d.ap()[2][0:64, :])
        nc.sync.dma_start(wall[8][64:128, :], bv_d.ap()[3][0:64, :])

        # ================= eff patterns (rank-1 K=1 matmuls) ===============
        # effA[d,e] = sum_i w_i * (d < hs_i) * (e < emb_i); light variant
        # effB over configs {1,3} uses the (d%64 < hs) pattern. All row
        # patterns live on partition 0 (engine partition-offset rule) in two
        # consolidated tiles: u4 blocks 0-3 heavy, 4-5 light; vrow blocks
        # emb=256 / emb=512.
        effA = const_p.tile([128, C], FP32, tag="effA")
        effB = const_p.tile([128, C], FP32, tag="effB")
        u4 = stage_p.tile([1, 6 * 128], BF16, name="u4", tag="u4", bufs=1)
        nc.vector.memset(u4[:], 0.0)
        nc.vector.memset(u4[0:1, 0:64], 1.0)        # cfg0 hs=64
        nc.vector.memset(u4[0:1, 128:160], 1.0)    # cfg1 hs=32
        nc.vector.memset(u4[0:1, 256:384], 1.0)    # cfg2 hs=128
        nc.vector.memset(u4[0:1, 384:448], 1.0)    # cfg3 hs=64
        nc.vector.memset(u4[0:1, 512:544], 1.0)    # light cfg1: d%64<32
        nc.vector.memset(u4[0:1, 576:608], 1.0)
        nc.vector.memset(u4[0:1, 640:768], 1.0)    # light cfg3: all
        vrow = stage_p.tile([1, 2 * C], BF16, name="vrow", tag="vrow", bufs=1)
        nc.vector.memset(vrow[:], 0.0)
        nc.vector.memset(vrow[0:1, 0:256], 1.0)    # emb=256 pattern
        nc.vector.memset(vrow[0:1, C:2 * C], 1.0)  # emb=512 pattern
        uw4 = stage_p.tile([1, 6 * 128], BF16, name="uw4", tag="uw4", bufs=1)
        for i in range(6):
            wi = (0, 1, 2, 3, 1, 3)[i]
            nc.vector.tensor_scalar_mul(
                uw4[0:1, i * 128:(i + 1) * 128],
                u4[0:1, i * 128:(i + 1) * 128], w_row[0:1, wi:wi + 1])
        EMBSEL = (0, 0, 1, 1, 0, 1)  # which vrow block per config
        pE = ps.tile([128, 512], FP32, name="ps", tag="ps")
        for n, i in enumerate((0, 1, 2, 3)):
            nc.tensor.matmul(pE[:], uw4[0:1, i * 128:(i + 1) * 128],
                             vrow[0:1, EMBSEL[i] * C:(EMBSEL[i] + 1) * C],
                             start=(n == 0), stop=(n == 3))
        nc.vector.tensor_copy(effA[:], pE[:])
        pE2 = ps.tile([128, 512], FP32, name="ps", tag="ps")
        for n, i in enumerate((4, 5)):
            nc.tensor.matmul(pE2[:], uw4[0:1, i * 128:(i + 1) * 128],
                             vrow[0:1, EMBSEL[i] * C:(EMBSEL[i] + 1) * C],
                             start=(n == 0), stop=(n == 1))
        nc.vector.tensor_copy(effB[:], pE2[:])

        # ================= persistent compute tensors ====================
        xt_all = xt_p.tile([128, NCC * T], BF16, tag="xt_all")
        xt = [xt_all[:, cc * T:(cc + 1) * T] for cc in range(NCC)]
        xt3 = xt_all[:].rearrange("p (a t) -> p a t", a=NCC)

        wtt9 = wts_p.tile([128, 9 * 512], BF16, tag="wtt9")
        wtt = [wtt9[:, j * 512:(j + 1) * 512] for j in range(9)]
        wt = [[wtt9[:, j * 512 + cc * 128:j * 512 + (cc + 1) * 128]
               for cc in range(NCC)] for j in range(9)]
        wpair3 = wtt9[:].rearrange("p (j q) -> p j q", q=512)

        qt = [qk_p.tile([128, T], BF16, name=f"qt{h}", tag=f"qt{h}") for h in range(3)]
        kt = [qk_p.tile([128, T], BF16, name=f"kt{h}", tag=f"kt{h}") for h in range(3)]
        # vall[h]: 16 s-chunks of 132 cols. heavy: [v(0:128) | ones(128)];
        # light packs [v_l0(0:64) | ones(64) | v_l1(65:129) | ones(129)].
        # ones columns are set once via strided memsets and never rewritten.
        vall = [v_p.tile([128, NT128 * 132], BF16, name=f"vall{h}", tag=f"vall{h}")
                for h in range(3)]
        vall3 = [vall[h][:].rearrange("p (i c) -> p i c", c=132) for h in range(3)]
        nc.gpsimd.memset(vall3[0][:, :, 128:129], 1.0)
        nc.gpsimd.memset(vall3[1][:, :, 128:129], 1.0)
        nc.gpsimd.memset(vall3[2][:, :, 64:65], 1.0)
        nc.gpsimd.memset(vall3[2][:, :, 129:130], 1.0)

        pts = [pt_p.tile([128, (4 * tj + 4) * 512], BF16, name=f"pt{tj}", tag=f"pt{tj}")
               for tj in range(NT512)]

        # (proj unit, light half, v lo, v hi, out col)
        AU = [
            (0, None, 0, 129, 0),
            (1, None, 0, 129, 128),
            (2, 0, 0, 65, 256),
            (2, 1, 65, 130, 384),
        ]

        def kt_ap(au_i, i):
            pu, half = AU[au_i][0], AU[au_i][1]
            sl = slice(i * 128, (i + 1) * 128)
            if half is None:
                return kt[pu][:, sl]
            return kt[2][64 * half:64 * half + 64, sl]

        def qt_ap(au_i, tj):
            pu, half = AU[au_i][0], AU[au_i][1]
            sl = slice(tj * 512, (tj + 1) * 512)
            if half is None:
                return qt[pu][:, sl]
            return qt[2][64 * half:64 * half + 64, sl]

        # ================= work items (filler queue) =====================
        def xprep(tt):
            def fn():
                # fp32 transpose straight off the DMA; the PSUM->SBUF
                # consolidation does the bf16 cast. During startup ACT is
                # idle, so the first four consolidations run there instead
                # of on the (startup-critical) DVE.
                ptp = ps.tile([128, 512], FP32, name="ps", tag="ps")
                for cc in range(NCC):
                    nc.tensor.transpose(
                        ptp[:, cc * 128:(cc + 1) * 128],
                        xall[:, tt * C + cc * 128:tt * C + (cc + 1) * 128],
                        ident_f[:])
                nc.vector.tensor_copy(
                    xt3[:, :, tt * 128:(tt + 1) * 128],
                    ptp[:].rearrange("p (a t) -> p a t", a=NCC))
            return (700, fn)

        def witem(j):
            def fn():
                w_bf = stage_p.tile([128, C], BF16, name="w_bf", tag="w_bf", bufs=2)
                nc.vector.tensor_mul(w_bf[:], wall[j][:],
                                     effB[:] if (j % 3) == 2 else effA[:])
                ptp = ps.tile([128, 512], BF16, name="ps", tag="ps")
                for cc in range(NCC):
                    nc.tensor.transpose(
                        ptp[:, cc * 128:(cc + 1) * 128],
                        w_bf[:, cc * 128:(cc + 1) * 128], ident_b[:])
                cp = nc.scalar.copy if j in (0, 3) else nc.vector.tensor_copy
                cp(wtt[j], ptp[:])
            return (400, fn)

        def qkitem(pu, tj, which):  # which: 0 = q, 1 = k
            def fn():
                j = 3 * which + pu
                dst = (qt if which == 0 else kt)[pu]
                p = ps.tile([128, 512], FP32, name="ps", tag="ps")
                for cc in range(NCC):
                    nc.tensor.matmul(
                        p[:], wt[j][cc][:], xt[cc][:, tj * 512:(tj + 1) * 512],
                        start=(cc == 0), stop=(cc == NCC - 1))
                cp = (nc.scalar.copy if (pu == 0 and tj == 0)
                      else nc.vector.tensor_copy)
                cp(dst[:, tj * 512:(tj + 1) * 512], p[:])
            return (1000, fn)

        def vitem(pu, i):
            def fn():
                p = ps.tile([128, 512], FP32, name="ps", tag="ps")
                if pu == 0:
                    # both heavy heads' v in one 256-col moving pass
                    for cc in range(NCC):
                        nc.tensor.matmul(
                            p[:, 0:256], xt[cc][:, i * 128:(i + 1) * 128],
                            wpair3[:, 6:8, cc * 128:(cc + 1) * 128],
                            start=(cc == 0), stop=(cc == NCC - 1))
                    nc.vector.tensor_copy(
                        vall[0][:, i * 132:i * 132 + 128], p[:, 0:128])
                    nc.vector.tensor_copy(
                        vall[1][:, i * 132:i * 132 + 128], p[:, 128:256])
                else:
                    for cc in range(NCC):
                        nc.tensor.matmul(
                            p[:, 0:128], xt[cc][:, i * 128:(i + 1) * 128],
                            wt[6 + pu][cc][:],
                            start=(cc == 0), stop=(cc == NCC - 1))
                    vt_i = vall[pu][:, i * 132:(i + 1) * 132]
                    nc.vector.tensor_copy(vt_i[:, 0:64], p[:, 0:64])
                    nc.vector.tensor_copy(vt_i[:, 65:129], p[:, 64:128])
            return (550 if pu == 0 else 350, fn)

        # per-(unit, t-chunk) output stage: 4 normalized m-blocks gathered
        # into one SBUF tile, stored with a single strided DMA.
        out16 = out_d.ap().rearrange("(i p) c -> p i c", p=128)
        ob4_cur = {}

        def avitem(au_i, tj, m):
            pu, half, v_lo, v_hi, ocol = AU[au_i]
            w = v_hi - v_lo
            wm = w - 1
            ti = 4 * tj + m

            def fn():
                op = pav.tile([128, 256], FP32, name="pav", tag="pav")
                ptile = pts[tj]
                for i in range(ti + 1):
                    nc.tensor.matmul(
                        op[:, 0:w],
                        ptile[:, i * 512 + m * 128: i * 512 + (m + 1) * 128],
                        vall[pu][:, i * 132 + v_lo:i * 132 + v_hi],
                        start=(i == 0), stop=(i == ti))
                osb = osb_p.tile([128, 129], FP32, name="osb", tag="osb")
                nc.vector.tensor_copy(osb[:, 0:w], op[:, 0:w])
                if m == 0:
                    tag = "ob4h" if wm == 128 else "ob4l"
                    ob4_cur[au_i] = o_p.tile([128, 4 * wm], FP32,
                                             name=tag, tag=tag, bufs=3)
                ob4 = ob4_cur[au_i]
                nc.gpsimd.normalize_recip(
                    ob4[:, m * wm:(m + 1) * wm], osb[:, 0:wm], osb[:, wm:w])
                if m == 3:
                    nc.sync.dma_start(
                        out16[:, 4 * tj:4 * tj + 4, ocol:ocol + wm],
                        ob4[:].rearrange("p (m c) -> p m c", m=4))
            return ((ti + 1) * w * 0.46 + 250, fn)

        def spinitem():
            def fn():
                sp0 = ps.tile([128, 512], FP32, name="ps", tag="ps")
                nc.tensor.matmul(sp0[:], ones_b[:, 0:128], ones_b[:],
                                 start=True, stop=True)
            return (215, fn)

        # ---- filler queue: emitted into PE gaps while ACT drains exp ----
        # Items are (deadline_chunk, cost_ns, fn). FIFO; deadlines force
        # emission before the consumer chunk starts (Tile resolves data deps
        # by emission order, so a producer must be emitted before its reader).
        fill = []

        def drain(budget):
            spent = 0.0
            while fill and spent < budget:
                _, cost, fn = fill.pop(0)
                fn()
                spent += cost

        def drain_due(ci):
            # emit every due item, preserving relative queue order (producer
            # deadlines are always <= their consumers', so this is dep-safe)
            due = [it for it in fill if it[0] <= ci]
            if due:
                fill[:] = [it for it in fill if it[0] > ci]
                for it in due:
                    it[2]()

        def drain_all():
            while fill:
                fill.pop(0)[2]()

        # ================= score chunks with budgeted filler ==============
        def sched_scores(au_i, tj, final=False):
            S = 4 * tj + 4
            ptile = pts[tj]
            g = 0
            done_av = 0
            while g < S:
                gw = min(SGRP, S - g)
                sp = sps.tile([128, SGRP * 512], FP32, name="sps", tag="sps")
                for k in range(gw):
                    i = g + k
                    nc.tensor.matmul(sp[:, k * 512:(k + 1) * 512],
                                     kt_ap(au_i, i), qt_ap(au_i, tj),
                                     start=True, stop=True)
                nc.scalar.activation(
                    ptile[:, g * 512:(g + gw) * 512],
                    sp[:, 0:gw * 512], AF.Exp, scale=SCALE)
                g += gw
                if final:
                    # last chunk (tj == 0): mask + AV per diagonal block as
                    # soon as its exp lands, so the tail is a single m-block
                    while done_av < min(g, 4):
                        r = done_av
                        blk = ptile[:, r * 512 + r * 128:
                                    r * 512 + (r + 1) * 128]
                        nc.vector.tensor_mul(
                            blk, blk, tri4[:, r * 128:(r + 1) * 128])
                        avitem(au_i, tj, r)[1]()
                        done_av += 1
                else:
                    # ACT time of this group minus our own PE time
                    drain(gw * 427 + 470 - gw * 240)
            if not final:
                # causal tri on the 4 diagonal 128-blocks, one fused op:
                # block r sits at 128-col index 16*tj + 5*r (stride 5)
                dst = ptile[:].rearrange("p (i q) -> p i q", q=128)[
                    :, 16 * tj:16 * tj + 16:5, :]
                nc.vector.tensor_mul(dst, dst, tri3)

        # ================= main pipeline ==================================
        # HAM warm-up: keep PE streaming through startup DMA waits so the
        # clock gate reaches 8/8 before the first score chunk.
        def spin(n):
            for _ in range(n):
                sp0 = ps.tile([128, 512], FP32, name="ps", tag="ps")
                nc.tensor.matmul(sp0[:], ones_b[:, 0:128], ones_b[:],
                                 start=True, stop=True)

        # startup: weights q0/k0, x chunk 0, unit-0 qk proj for t-chunk 0
        spin(4)
        witem(0)[1]()
        witem(3)[1]()
        spin(4)
        for tt in range(4):
            xprep(tt)[1]()
        spin(2)
        qkitem(0, 0, 0)[1]()
        qkitem(0, 0, 1)[1]()

        # per-chunk filler: startup leftovers + next-unit projections + AV.
        # E(dl, item) attaches the mandatory-emission deadline chunk.
        def E(dl, item):
            return (dl, item[0], item[1])

        extras = {c: [] for c in range(16)}
        for _ in range(3):
            extras[0].append(E(99, spinitem()))
            extras[1].append(E(99, spinitem()))
        extras[0] += [E(1, witem(6)),
                      E(1, vitem(0, 0)), E(1, vitem(0, 1)),
                      E(1, vitem(0, 2)), E(1, vitem(0, 3)),
                      E(1, xprep(4)), E(1, xprep(5)),
                      E(1, xprep(6)), E(1, xprep(7)),
                      E(1, qkitem(0, 1, 0)), E(1, qkitem(0, 1, 1))]
        extras[1] += [E(2, xprep(8)), E(2, xprep(9)),
                      E(2, xprep(10)), E(2, xprep(11)),
                      E(2, qkitem(0, 2, 0)), E(2, qkitem(0, 2, 1)),
                      E(2, vitem(0, 4)), E(2, vitem(0, 5)),
                      E(2, vitem(0, 6)), E(2, vitem(0, 7)),
                      E(4, witem(1)), E(4, witem(4))]
        extras[2] += [E(3, xprep(12)), E(3, xprep(13)),
                      E(3, xprep(14)), E(3, xprep(15)),
                      E(3, qkitem(0, 3, 0)), E(3, qkitem(0, 3, 1)),
                      E(3, vitem(0, 8)), E(3, vitem(0, 9)),
                      E(3, vitem(0, 10)), E(3, vitem(0, 11)),
                      E(5, witem(7))]
        extras[3] += [E(4, vitem(0, 12)), E(4, vitem(0, 13)),
                      E(4, vitem(0, 14)), E(4, vitem(0, 15)),
                      E(8, witem(2)), E(8, witem(5)), E(8, witem(8))]
        # unit-0 windows carry proj of unit 1; unit-1 windows carry the
        # packed-light projections (units 2 and 3 share proj unit 2).
        for u in range(2):
            pu = u + 1
            base_c = 4 * pu
            for tj in range(NT512):
                c = 4 * u + tj
                extras[c] += [E(base_c + tj, qkitem(pu, tj, 0)),
                              E(base_c + tj, qkitem(pu, tj, 1))]
                if pu == 2:  # light v; heavy v pair is computed with pu=0
                    extras[c] += [E(base_c + 1 + tj, vitem(pu, 4 * tj + r))
                                  for r in range(4)]

        # unit 3 runs [2,3,1,0] so the tail AV after the last scores chunk
        # is the small tj=0 one (10 matmuls) instead of tj=3 (58).
        CH = ([(u, t) for u in range(3) for t in range(4)]
              + [(3, 2), (3, 3), (3, 1), (3, 0)])

        def av_deadline(ci, ptj):
            # AV of (pau, ptj) must be emitted before exp next rewrites
            # pts[ptj]; drain_due runs before sched_scores, so deadline may
            # equal that chunk's index.
            for c2 in range(ci, len(CH)):
                if CH[c2][1] == ptj:
                    return min(ci + 1, c2)
            return ci + 1

        for ci, (au_i, tj) in enumerate(CH):
            last = ci == len(CH) - 1
            if ci > 0:
                pau, ptj = CH[ci - 1]
                dl = ci if last else av_deadline(ci, ptj)
                for m in range(4):
                    fill.append(E(dl, avitem(pau, ptj, m)))
            fill.extend(extras[ci])
            drain_due(ci)
            sched_scores(au_i, tj, final=last)
        drain_all()


def _shard_inputs(x, weights, base_K, base_Q, base_V):
    in_maps = []
    for c in range(8):
        b = c // 2
        hsel = [0, 1, 4, 5] if c % 2 == 0 else [2, 3, 6, 7]
        in_maps.append({
            "x": np.ascontiguousarray(x[b]),
            "w": np.ascontiguousarray(weights.reshape(4, 1)),
            "bq": np.ascontiguousarray(base_Q[hsel]),
            "bk": np.ascontiguousarray(base_K[hsel]),
            "bv": np.ascontiguousarray(base_V[hsel]),
        })
    return in_maps


def _gather(results):
    out = np.zeros((4, T, 8 * HS), np.float32)
    for c in range(8):
        o = results[c]["out"]
        hsel = [0, 1, 4, 5] if c % 2 == 0 else [2, 3, 6, 7]
        for j, h in enumerate(hsel):
            out[c // 2][:, h * HS:(h + 1) * HS] = o[:, j * HS:(j + 1) * HS]
    return out


def get_nc():
    if "nc" not in _CACHE:
        _CACHE["nc"] = _build()
    return _CACHE["nc"]


def kernel(x, weights, base_K, base_Q, base_V):
    x = np.asarray(x, np.float32)
    weights = np.asarray(weights, np.float32)
    base_K = np.asarray(base_K, np.float32)
    base_Q = np.asarray(base_Q, np.float32)
    base_V = np.asarray(base_V, np.float32)
    nc = get_nc()
    in_maps = _shard_inputs(x, weights, base_K, base_Q, base_V)
    res = run_bass_kernel_spmd(nc, in_maps, core_ids=list(range(8)))
    return _gather(res.results)


# revision 37
# speedup vs baseline: 1.1149x; 1.0351x over previous
"""Trainium2 Bass kernel for nn_MixedHeadsV2 (mixed-head causal attention).

Full inputs in, full output out. Sharding: 8 cores = 4 batches x 2 head-groups.
Each core handles one batch and 4 of the 8 base heads: even cores heads
{0,1,4,5}, odd cores {2,3,6,7}. Heads 0-3 ("heavy") have effective head size
128; heads 4-7 ("light") have effective head size 64 (their mixed weight rows
64:128 are exactly zero), so the two light heads are packed into one 128-wide
tensor for projections and use 64-partition (K=64) score matmuls.

v2 engine plan (per core, Tile-scheduled, all instruction streams
software-pipelined so ACT/exp never starves and PE never idles):
  PE:   fp32 x-transposes, bf16 W-transposes, projections, scores, AV.
        AV of t-chunk tj is emitted during scores of chunk tj+1 (delay-1
        software pipeline); projections of the next unit and the x/W prep
        fill remaining PE slack via a cost-budgeted filler queue.
  ACT:  exp ONLY (scale folded), groups of 3 score tiles [128,1536] from a
        2-buf x 3-bank PSUM pool.
  DVE:  PSUM->SBUF glue (x consolidate+cast, qk casts, v copies, AV copies),
        fused 2-block causal tri masks.
  Pool: normalize_recip (out = av/denom, SBUF-side), vtile ones init,
        weight-DMA issue, affine_select consts.
  Sync: x loads + output stores.
"""
import sys

for p in ("/opt/trn_rl_repo",):
    if p not in sys.path:
        sys.path.append(p)

import numpy as np

import concourse.bass as bass
import concourse.tile as tile
from concourse import bacc, mybir
from concourse.bass_utils import run_bass_kernel_spmd

FP32 = mybir.dt.float32
BF16 = mybir.dt.bfloat16
AF = mybir.ActivationFunctionType
ALU = mybir.AluOpType

T = 2048
C = 512
HS = 128          # heavy head size (= padded head size)
NT128 = T // 128  # 16
NT512 = T // 512  # 4
NCC = C // 128    # 4
SCALE = float(1.0 / np.sqrt(128.0))
SGRP = 2          # score tiles (512 wide) per exp group / PSUM banks per buf

_CACHE = {}


def _build():
    nc = bacc.Bacc("TRN2", target_bir_lowering=False, debug=False, num_devices=8)
    x_d = nc.dram_tensor("x", [T, C], FP32, kind="ExternalInput")
    w_d = nc.dram_tensor("w", [4, 1], FP32, kind="ExternalInput")
    bq_d = nc.dram_tensor("bq", [4, HS, C], FP32, kind="ExternalInput")
    bk_d = nc.dram_tensor("bk", [4, HS, C], FP32, kind="ExternalInput")
    bv_d = nc.dram_tensor("bv", [4, HS, C], FP32, kind="ExternalInput")
    out_d = nc.dram_tensor("out", [T, 4 * HS], FP32, kind="ExternalOutput")

    with tile.TileContext(nc) as tc:
        _emit(nc, tc, x_d, w_d, bq_d, bk_d, bv_d, out_d)
    nc.compile()
    return nc


def _emit(nc, tc, x_d, w_d, bq_d, bk_d, bv_d, out_d):
    from contextlib import ExitStack

    ctx = ExitStack()
    with ctx:
        # ---- persistent SBUF pools ----
        const_p = ctx.enter_context(tc.tile_pool(name="const", bufs=1))
        wall_p = ctx.enter_context(tc.tile_pool(name="wall", bufs=1))
        wts_p = ctx.enter_context(tc.tile_pool(name="wts", bufs=1))
        xall_p = ctx.enter_context(tc.tile_pool(name="xall", bufs=1))
        xt_p = ctx.enter_context(tc.tile_pool(name="xt", bufs=1))
        qk_p = ctx.enter_context(tc.tile_pool(name="qk", bufs=1))
        v_p = ctx.enter_context(tc.tile_pool(name="v", bufs=1))
        pt_p = ctx.enter_context(tc.tile_pool(name="pt", bufs=1))
        osb_p = ctx.enter_context(tc.tile_pool(name="osb", bufs=8))
        o_p = ctx.enter_context(tc.tile_pool(name="o", bufs=4))
        stage_p = ctx.enter_context(tc.tile_pool(name="stage", bufs=2))
        # ---- PSUM: 2 bufs x 2 banks for score groups, 2 x AV-chain bufs,
        # 2 x 1 bank for projections/transposes (separate pools so an AV
        # chain never waits on a projection's PSUM->SBUF cast)
        sps = ctx.enter_context(tc.tile_pool(name="sps", bufs=2, space="PSUM"))
        pav = ctx.enter_context(tc.tile_pool(name="pav", bufs=2, space="PSUM"))
        ps = ctx.enter_context(tc.tile_pool(name="ps", bufs=2, space="PSUM"))

        # ================= constants (first: Pool/DVE free, no DMA deps) ===
        ones_b = const_p.tile([128, C], BF16, tag="ones_b")
        nc.vector.memset(ones_b[:], 1.0)
        ident_b = const_p.tile([128, 128], BF16, tag="ident_b")
        nc.gpsimd.affine_select(
            ident_b[:], ones_b[:, 0:128], pattern=[[1, 128]],
            compare_op=ALU.is_equal, fill=0.0, base=0, channel_multiplier=-1)
        ident_f = const_p.tile([128, 128], FP32, tag="ident_f")
        nc.vector.tensor_copy(ident_f[:], ident_b[:])
        # causal triangle x4: tri4[:, r*128+t] = (t >= s) for r = 0..3
        tri4 = const_p.tile([128, 512], BF16, tag="tri4")
        for r in range(4):
            nc.gpsimd.affine_select(
                tri4[:, r * 128:(r + 1) * 128], ones_b[:, 0:128],
                pattern=[[1, 128]], compare_op=ALU.is_ge, fill=0.0, base=0,
                channel_multiplier=-1)
        tri3 = tri4[:].rearrange("p (r q) -> p r q", r=4)
        # start the PE streaming immediately (HAM clock-gate warm-up)
        for _ in range(3):
            sp0 = ps.tile([128, 512], FP32, name="ps", tag="ps")
            nc.tensor.matmul(sp0[:], ones_b[:, 0:128], ones_b[:],
                             start=True, stop=True)

        # ================= DMA issue (transfers run in background) =========
        # Three DMA rings in parallel so the startup critical path is
        # max(x chunk 0, q0/k0 bases) instead of their sum:
        #   sync:   x (first 4 tiles singly for early readiness, then groups),
        #           late v bases
        #   pool:   w_row + near-term weight bases
        #   scalar: light-unit q/k bases (ACT idle until first exp)
        w_row = const_p.tile([1, 4], FP32, tag="w_row")
        wall = [wall_p.tile([128, C], FP32, name=f"wall{j}", tag=f"wall{j}") for j in range(9)]
        xall = xall_p.tile([128, NT128 * C], FP32, tag="xall")
        xall3 = xall[:].rearrange("p (i c) -> p i c", c=C)
        x16 = x_d.ap().rearrange("(i p) c -> p i c", p=128)
        for tt in range(4):
            nc.sync.dma_start(xall3[:, tt:tt + 1, :], x16[:, tt:tt + 1, :])
        for grp in range(1, 4):
            nc.sync.dma_start(xall3[:, grp * 4:(grp + 1) * 4, :],
                              x16[:, grp * 4:(grp + 1) * 4, :])
        # j = 0..2 -> q(h0,h1,light), 3..5 -> k, 6..8 -> v; light packs
        # head2[0:64] + head3[0:64].
        nc.gpsimd.dma_start(w_row[:], w_d.ap().rearrange("a b -> b a"))
        nc.gpsimd.dma_start(wall[0][:], bq_d.ap()[0])
        nc.gpsimd.dma_start(wall[3][:], bk_d.ap()[0])
        nc.gpsimd.dma_start(wall[6][:], bv_d.ap()[0])
        nc.gpsimd.dma_start(wall[1][:], bq_d.ap()[1])
        nc.gpsimd.dma_start(wall[4][:], bk_d.ap()[1])
        nc.scalar.dma_start(wall[2][0:64, :], bq_d.ap()[2][0:64, :])
        nc.scalar.dma_start(wall[2][64:128, :], bq_d.ap()[3][0:64, :])
        nc.scalar.dma_start(wall[5][0:64, :], bk_d.ap()[2][0:64, :])
        nc.scalar.dma_start(wall[5][64:128, :], bk_d.ap()[3][0:64, :])
        nc.sync.dma_start(wall[7][:], bv_d.ap()[1])
        nc.sync.dma_start(wall[8][0:64, :], bv_d.ap()[2][0:64, :])
        nc.sync.dma_start(wall[8][64:128, :], bv_d.ap()[3][0:64, :])

        # ================= eff patterns (rank-1 K=1 matmuls) ===============
        # effA[d,e] = sum_i w_i * (d < hs_i) * (e < emb_i); light variant
        # effB over configs {1,3} uses the (d%64 < hs) pattern. All row
        # patterns live on partition 0 (engine partition-offset rule) in two
        # consolidated tiles: u4 blocks 0-3 heavy, 4-5 light; vrow blocks
        # emb=256 / emb=512.
        effA = const_p.tile([128, C], FP32, tag="effA")
        effB = const_p.tile([128, C], FP32, tag="effB")
        u4 = stage_p.tile([1, 6 * 128], BF16, name="u4", tag="u4", bufs=1)
        nc.vector.memset(u4[:], 0.0)
        nc.vector.memset(u4[0:1, 0:64], 1.0)        # cfg0 hs=64
        nc.vector.memset(u4[0:1, 128:160], 1.0)    # cfg1 hs=32
        nc.vector.memset(u4[0:1, 256:384], 1.0)    # cfg2 hs=128
        nc.vector.memset(u4[0:1, 384:448], 1.0)    # cfg3 hs=64
        nc.vector.memset(u4[0:1, 512:544], 1.0)    # light cfg1: d%64<32
        nc.vector.memset(u4[0:1, 576:608], 1.0)
        nc.vector.memset(u4[0:1, 640:768], 1.0)    # light cfg3: all
        vrow = stage_p.tile([1, 2 * C], BF16, name="vrow", tag="vrow", bufs=1)
        nc.vector.memset(vrow[:], 0.0)
        nc.vector.memset(vrow[0:1, 0:256], 1.0)    # emb=256 pattern
        nc.vector.memset(vrow[0:1, C:2 * C], 1.0)  # emb=512 pattern
        uw4 = stage_p.tile([1, 6 * 128], BF16, name="uw4", tag="uw4", bufs=1)
        for i in range(6):
            wi = (0, 1, 2, 3, 1, 3)[i]
            nc.vector.tensor_scalar_mul(
                uw4[0:1, i * 128:(i + 1) * 128],
                u4[0:1, i * 128:(i + 1) * 128], w_row[0:1, wi:wi + 1])
        EMBSEL = (0, 0, 1, 1, 0, 1)  # which vrow block per config
        pE = ps.tile([128, 512], FP32, name="ps", tag="ps")
        for n, i in enumerate((0, 1, 2, 3)):
            nc.tensor.matmul(pE[:], uw4[0:1, i * 128:(i + 1) * 128],
                             vrow[0:1, EMBSEL[i] * C:(EMBSEL[i] + 1) * C],
                             start=(n == 0), stop=(n == 3))
        nc.vector.tensor_copy(effA[:], pE[:])
        pE2 = ps.tile([128, 512], FP32, name="ps", tag="ps")
        for n, i in enumerate((4, 5)):
            nc.tensor.matmul(pE2[:], uw4[0:1, i * 128:(i + 1) * 128],
                             vrow[0:1, EMBSEL[i] * C:(EMBSEL[i] + 1) * C],
                             start=(n == 0), stop=(n == 1))
        nc.vector.tensor_copy(effB[:], pE2[:])

        # ================= persistent compute tensors ====================
        xt_all = xt_p.tile([128, NCC * T], BF16, tag="xt_all")
        xt = [xt_all[:, cc * T:(cc + 1) * T] for cc in range(NCC)]
        xt3 = xt_all[:].rearrange("p (a t) -> p a t", a=NCC)

        wtt9 = wts_p.tile([128, 9 * 512], BF16, tag="wtt9")
        wtt = [wtt9[:, j * 512:(j + 1) * 512] for j in range(9)]
        wt = [[wtt9[:, j * 512 + cc * 128:j * 512 + (cc + 1) * 128]
               for cc in range(NCC)] for j in range(9)]
        wpair3 = wtt9[:].rearrange("p (j q) -> p j q", q=512)

        qt = [qk_p.tile([128, T], BF16, name=f"qt{h}", tag=f"qt{h}") for h in range(3)]
        kt = [qk_p.tile([128, T], BF16, name=f"kt{h}", tag=f"kt{h}") for h in range(3)]
        # vall[h]: 16 s-chunks of 132 cols. heavy: [v(0:128) | ones(128)];
        # light packs [v_l0(0:64) | ones(64) | v_l1(65:129) | ones(129)].
        # ones columns are set once via strided memsets and never rewritten.
        vall = [v_p.tile([128, NT128 * 132], BF16, name=f"vall{h}", tag=f"vall{h}")
                for h in range(3)]
        vall3 = [vall[h][:].rearrange("p (i c) -> p i c", c=132) for h in range(3)]
        nc.gpsimd.memset(vall3[0][:, :, 128:129], 1.0)
        nc.gpsimd.memset(vall3[1][:, :, 128:129], 1.0)
        nc.gpsimd.memset(vall3[2][:, :, 64:65], 1.0)
        nc.gpsimd.memset(vall3[2][:, :, 129:130], 1.0)

        pts = [pt_p.tile([128, (4 * tj + 4) * 512], BF16, name=f"pt{tj}", tag=f"pt{tj}")
               for tj in range(NT512)]

        # (proj unit, light half, v lo, v hi, out col)
        AU = [
            (0, None, 0, 129, 0),
            (1, None, 0, 129, 128),
            (2, 0, 0, 65, 256),
            (2, 1, 65, 130, 384),
        ]

        def kt_ap(au_i, i):
            pu, half = AU[au_i][0], AU[au_i][1]
            sl = slice(i * 128, (i + 1) * 128)
            if half is None:
                return kt[pu][:, sl]
            return kt[2][64 * half:64 * half + 64, sl]

        def qt_ap(au_i, tj):
            pu, half = AU[au_i][0], AU[au_i][1]
            sl = slice(tj * 512, (tj + 1) * 512)
            if half is None:
                return qt[pu][:, sl]
            return qt[2][64 * half:64 * half + 64, sl]

        # ================= work items (filler queue) =====================
        def xprep(tt):
            def fn():
                # fp32 transpose straight off the DMA; the PSUM->SBUF
                # consolidation does the bf16 cast. During startup ACT is
                # idle, so the first four consolidations run there instead
                # of on the (startup-critical) DVE.
                ptp = ps.tile([128, 512], FP32, name="ps", tag="ps")
                for cc in range(NCC):
                    nc.tensor.transpose(
                        ptp[:, cc * 128:(cc + 1) * 128],
                        xall[:, tt * C + cc * 128:tt * C + (cc + 1) * 128],
                        ident_f[:])
                nc.vector.tensor_copy(
                    xt3[:, :, tt * 128:(tt + 1) * 128],
                    ptp[:].rearrange("p (a t) -> p a t", a=NCC))
            return (700, fn)

        def witem(j):
            def fn():
                w_bf = stage_p.tile([128, C], BF16, name="w_bf", tag="w_bf", bufs=2)
                nc.vector.tensor_mul(w_bf[:], wall[j][:],
                                     effB[:] if (j % 3) == 2 else effA[:])
                ptp = ps.tile([128, 512], BF16, name="ps", tag="ps")
                for cc in range(NCC):
                    nc.tensor.transpose(
                        ptp[:, cc * 128:(cc + 1) * 128],
                        w_bf[:, cc * 128:(cc + 1) * 128], ident_b[:])
                cp = nc.scalar.copy if j in (0, 3) else nc.vector.tensor_copy
                cp(wtt[j], ptp[:])
            return (400, fn)

        def qkitem(pu, tj, which):  # which: 0 = q, 1 = k
            def fn():
                j = 3 * which + pu
                dst = (qt if which == 0 else kt)[pu]
                p = ps.tile([128, 512], FP32, name="ps", tag="ps")
                for cc in range(NCC):
                    nc.tensor.matmul(
                        p[:], wt[j][cc][:], xt[cc][:, tj * 512:(tj + 1) * 512],
                        start=(cc == 0), stop=(cc == NCC - 1))
                cp = (nc.scalar.copy if (pu == 0 and tj == 0)
                      else nc.vector.tensor_copy)
                cp(dst[:, tj * 512:(tj + 1) * 512], p[:])
            return (1000, fn)

        def vitem(pu, i):
            def fn():
                p = ps.tile([128, 512], FP32, name="ps", tag="ps")
                if pu == 0:
                    # both heavy heads' v in one 256-col moving pass
                    for cc in range(NCC):
                        nc.tensor.matmul(
                            p[:, 0:256], xt[cc][:, i * 128:(i + 1) * 128],
                            wpair3[:, 6:8, cc * 128:(cc + 1) * 128],
                            start=(cc == 0), stop=(cc == NCC - 1))
                    nc.vector.tensor_copy(
                        vall[0][:, i * 132:i * 132 + 128], p[:, 0:128])
                    nc.vector.tensor_copy(
                        vall[1][:, i * 132:i * 132 + 128], p[:, 128:256])
                else:
                    for cc in range(NCC):
                        nc.tensor.matmul(
                            p[:, 0:128], xt[cc][:, i * 128:(i + 1) * 128],
                            wt[6 + pu][cc][:],
                            start=(cc == 0), stop=(cc == NCC - 1))
                    vt_i = vall[pu][:, i * 132:(i + 1) * 132]
                    nc.vector.tensor_copy(vt_i[:, 0:64], p[:, 0:64])
                    nc.vector.tensor_copy(vt_i[:, 65:129], p[:, 64:128])
            return (550 if pu == 0 else 350, fn)

        # per-(unit, t-chunk) output stage: 4 normalized m-blocks gathered
        # into one SBUF tile, stored with a single strided DMA.
        out16 = out_d.ap().rearrange("(i p) c -> p i c", p=128)
        ob4_cur = {}

        def avitem(au_i, tj, m):
            pu, half, v_lo, v_hi, ocol = AU[au_i]
            w = v_hi - v_lo
            wm = w - 1
            ti = 4 * tj + m

            def fn():
                op = pav.tile([128, 256], FP32, name="pav", tag="pav")
                ptile = pts[tj]
                for i in range(ti + 1):
                    nc.tensor.matmul(
                        op[:, 0:w],
                        ptile[:, i * 512 + m * 128: i * 512 + (m + 1) * 128],
                        vall[pu][:, i * 132 + v_lo:i * 132 + v_hi],
                        start=(i == 0), stop=(i == ti))
                osb = osb_p.tile([128, 129], FP32, name="osb", tag="osb")
                nc.vector.tensor_copy(osb[:, 0:w], op[:, 0:w])
                if m == 0:
                    tag = "ob4h" if wm == 128 else "ob4l"
                    ob4_cur[au_i] = o_p.tile([128, 4 * wm], FP32,
                                             name=tag, tag=tag, bufs=3)
                ob4 = ob4_cur[au_i]
                nc.gpsimd.normalize_recip(
                    ob4[:, m * wm:(m + 1) * wm], osb[:, 0:wm], osb[:, wm:w])
                if m == 3:
                    nc.sync.dma_start(
                        out16[:, 4 * tj:4 * tj + 4, ocol:ocol + wm],
                        ob4[:].rearrange("p (m c) -> p m c", m=4))
            return ((ti + 1) * w * 0.46 + 250, fn)

        def spinitem():
            def fn():
                sp0 = ps.tile([128, 512], FP32, name="ps", tag="ps")
                nc.tensor.matmul(sp0[:], ones_b[:, 0:128], ones_b[:],
                                 start=True, stop=True)
            return (215, fn)

        # ---- filler queue: emitted into PE gaps while ACT drains exp ----
        # Items are (deadline_chunk, cost_ns, fn). FIFO; deadlines force
        # emission before the consumer chunk starts (Tile resolves data deps
        # by emission order, so a producer must be emitted before its reader).
        fill = []

        def drain(budget):
            spent = 0.0
            while fill and spent < budget:
                _, cost, fn = fill.pop(0)
                fn()
                spent += cost

        def drain_due(ci):
            # emit every due item, preserving relative queue order (producer
            # deadlines are always <= their consumers', so this is dep-safe)
            due = [it for it in fill if it[0] <= ci]
            if due:
                fill[:] = [it for it in fill if it[0] > ci]
                for it in due:
                    it[2]()

        def drain_all():
            while fill:
                fill.pop(0)[2]()

        # ================= score chunks with budgeted filler ==============
        def sched_scores(au_i, tj, final=False):
            S = 4 * tj + 4
            ptile = pts[tj]
            g = 0
            done_av = 0
            while g < S:
                gw = min(SGRP, S - g)
                sp = sps.tile([128, SGRP * 512], FP32, name="sps", tag="sps")
                for k in range(gw):
                    i = g + k
                    # diagonal chunks (i = 4tj+r): t-cols below 128r are
                    # strictly above the causal boundary and never read
                    # downstream — skip them in the matmul (exp still runs
                    # full-width over stale PSUM there; harmless, unread)
                    lo = max(0, (i - 4 * tj) * 128)
                    qa = qt_ap(au_i, tj)
                    nc.tensor.matmul(sp[:, k * 512 + lo:(k + 1) * 512],
                                     kt_ap(au_i, i), qa[:, lo:512],
                                     start=True, stop=True)
                nc.scalar.activation(
                    ptile[:, g * 512:(g + gw) * 512],
                    sp[:, 0:gw * 512], AF.Exp, scale=SCALE)
                g += gw
                if final:
                    # last chunk (tj == 0): mask + AV per diagonal block as
                    # soon as its exp lands, so the tail is a single m-block
                    while done_av < min(g, 4):
                        r = done_av
                        blk = ptile[:, r * 512 + r * 128:
                                    r * 512 + (r + 1) * 128]
                        nc.vector.tensor_mul(
                            blk, blk, tri4[:, r * 128:(r + 1) * 128])
                        avitem(au_i, tj, r)[1]()
                        done_av += 1
                else:
                    # ACT time of this group minus our own PE time
                    drain(gw * 427 + 470 - gw * 240)
            if not final:
                # causal tri on the 4 diagonal 128-blocks, one fused op:
                # block r sits at 128-col index 16*tj + 5*r (stride 5)
                dst = ptile[:].rearrange("p (i q) -> p i q", q=128)[
                    :, 16 * tj:16 * tj + 16:5, :]
                nc.vector.tensor_mul(dst, dst, tri3)

        # ================= main pipeline ==================================
        # HAM warm-up: keep PE streaming through startup DMA waits so the
        # clock gate reaches 8/8 before the first score chunk.
        def spin(n):
            for _ in range(n):
                sp0 = ps.tile([128, 512], FP32, name="ps", tag="ps")
                nc.tensor.matmul(sp0[:], ones_b[:, 0:128], ones_b[:],
                                 start=True, stop=True)

        # startup: weights q0/k0, x chunk 0, unit-0 qk proj for t-chunk 0
        spin(4)
        witem(0)[1]()
        witem(3)[1]()
        spin(4)
        for tt in range(4):
            xprep(tt)[1]()
        spin(2)
        qkitem(0, 0, 0)[1]()
        qkitem(0, 0, 1)[1]()

        # per-chunk filler: startup leftovers + next-unit projections + AV.
        # E(dl, item) attaches the mandatory-emission deadline chunk.
        def E(dl, item):
            return (dl, item[0], item[1])

        extras = {c: [] for c in range(16)}
        for _ in range(3):
            extras[0].append(E(99, spinitem()))
            extras[1].append(E(99, spinitem()))
        extras[0] += [E(1, witem(6)),
                      E(1, vitem(0, 0)), E(1, vitem(0, 1)),
                      E(1, vitem(0, 2)), E(1, vitem(0, 3)),
                      E(1, xprep(4)), E(1, xprep(5)),
                      E(1, xprep(6)), E(1, xprep(7)),
                      E(1, qkitem(0, 1, 0)), E(1, qkitem(0, 1, 1))]
        extras[1] += [E(2, xprep(8)), E(2, xprep(9)),
                      E(2, xprep(10)), E(2, xprep(11)),
                      E(2, qkitem(0, 2, 0)), E(2, qkitem(0, 2, 1)),
                      E(2, vitem(0, 4)), E(2, vitem(0, 5)),
                      E(2, vitem(0, 6)), E(2, vitem(0, 7)),
                      E(4, witem(1)), E(4, witem(4))]
        extras[2] += [E(3, xprep(12)), E(3, xprep(13)),
                      E(3, xprep(14)), E(3, xprep(15)),
                      E(3, qkitem(0, 3, 0)), E(3, qkitem(0, 3, 1)),
                      E(3, vitem(0, 8)), E(3, vitem(0, 9)),
                      E(3, vitem(0, 10)), E(3, vitem(0, 11)),
                      E(5, witem(7))]
        extras[3] += [E(4, vitem(0, 12)), E(4, vitem(0, 13)),
                      E(4, vitem(0, 14)), E(4, vitem(0, 15)),
                      E(8, witem(2)), E(8, witem(5)), E(8, witem(8))]
        # unit-0 windows carry proj of unit 1; unit-1 windows carry the
        # packed-light projections (units 2 and 3 share proj unit 2).
        for u in range(2):
            pu = u + 1
            base_c = 4 * pu
            for tj in range(NT512):
                c = 4 * u + tj
                extras[c] += [E(base_c + tj, qkitem(pu, tj, 0)),
                              E(base_c + tj, qkitem(pu, tj, 1))]
                if pu == 2:  # light v; heavy v pair is computed with pu=0
                    extras[c] += [E(base_c + 1 + tj, vitem(pu, 4 * tj + r))
                                  for r in range(4)]

        # unit 3 runs [2,3,1,0] so the tail AV after the last scores chunk
        # is the small tj=0 one (10 matmuls) instead of tj=3 (58).
        CH = ([(u, t) for u in range(3) for t in range(4)]
              + [(3, 2), (3, 3), (3, 1), (3, 0)])

        def av_deadline(ci, ptj):
            # AV of (pau, ptj) must be emitted before exp next rewrites
            # pts[ptj]; drain_due runs before sched_scores, so deadline may
            # equal that chunk's index.
            for c2 in range(ci, len(CH)):
                if CH[c2][1] == ptj:
                    return min(ci + 1, c2)
            return ci + 1

        for ci, (au_i, tj) in enumerate(CH):
            last = ci == len(CH) - 1
            if ci > 0:
                pau, ptj = CH[ci - 1]
                dl = ci if last else av_deadline(ci, ptj)
                for m in range(4):
                    fill.append(E(dl, avitem(pau, ptj, m)))
            fill.extend(extras[ci])
            drain_due(ci)
            sched_scores(au_i, tj, final=last)
        drain_all()


def _shard_inputs(x, weights, base_K, base_Q, base_V):
    in_maps = []
    for c in range(8):
        b = c // 2
        hsel = [0, 1, 4, 5] if c % 2 == 0 else [2, 3, 6, 7]
        in_maps.append({
            "x": np.ascontiguousarray(x[b]),
            "w": np.ascontiguousarray(weights.reshape(4, 1)),
            "bq": np.ascontiguousarray(base_Q[hsel]),
            "bk": np.ascontiguousarray(base_K[hsel]),
            "bv": np.ascontiguousarray(base_V[hsel]),
        })
    return in_maps


def _gather(results):
    out = np.zeros((4, T, 8 * HS), np.float32)
    for c in range(8):
        o = results[c]["out"]
        hsel = [0, 1, 4, 5] if c % 2 == 0 else [2, 3, 6, 7]
        for j, h in enumerate(hsel):
            out[c // 2][:, h * HS:(h + 1) * HS] = o[:, j * HS:(j + 1) * HS]
    return out


def get_nc():
    if "nc" not in _CACHE:
        _CACHE["nc"] = _build()
    return _CACHE["nc"]


def kernel(x, weights, base_K, base_Q, base_V):
    x = np.asarray(x, np.float32)
    weights = np.asarray(weights, np.float32)
    base_K = np.asarray(base_K, np.float32)
    base_Q = np.asarray(base_Q, np.float32)
    base_V = np.asarray(base_V, np.float32)
    nc = get_nc()
    in_maps = _shard_inputs(x, weights, base_K, base_Q, base_V)
    res = run_bass_kernel_spmd(nc, in_maps, core_ids=list(range(8)))
    return _gather(res.results)


# revision 38
# speedup vs baseline: 1.1201x; 1.0046x over previous
"""Trainium2 Bass kernel for nn_MixedHeadsV2 (mixed-head causal attention).

Full inputs in, full output out. Sharding: 8 cores = 4 batches x 2 head-groups.
Each core handles one batch and 4 of the 8 base heads: even cores heads
{0,1,4,5}, odd cores {2,3,6,7}. Heads 0-3 ("heavy") have effective head size
128; heads 4-7 ("light") have effective head size 64 (their mixed weight rows
64:128 are exactly zero), so the two light heads are packed into one 128-wide
tensor for projections and use 64-partition (K=64) score matmuls.

v2 engine plan (per core, Tile-scheduled, all instruction streams
software-pipelined so ACT/exp never starves and PE never idles):
  PE:   fp32 x-transposes, bf16 W-transposes, projections, scores, AV.
        AV of t-chunk tj is emitted during scores of chunk tj+1 (delay-1
        software pipeline); projections of the next unit and the x/W prep
        fill remaining PE slack via a cost-budgeted filler queue.
  ACT:  exp ONLY (scale folded), groups of 3 score tiles [128,1536] from a
        2-buf x 3-bank PSUM pool.
  DVE:  PSUM->SBUF glue (x consolidate+cast, qk casts, v copies, AV copies),
        fused 2-block causal tri masks.
  Pool: normalize_recip (out = av/denom, SBUF-side), vtile ones init,
        weight-DMA issue, affine_select consts.
  Sync: x loads + output stores.
"""
import sys

for p in ("/opt/trn_rl_repo",):
    if p not in sys.path:
        sys.path.append(p)

import numpy as np

import concourse.bass as bass
import concourse.tile as tile
from concourse import bacc, mybir
from concourse.bass_utils import run_bass_kernel_spmd

FP32 = mybir.dt.float32
BF16 = mybir.dt.bfloat16
AF = mybir.ActivationFunctionType
ALU = mybir.AluOpType

T = 2048
C = 512
HS = 128          # heavy head size (= padded head size)
NT128 = T // 128  # 16
NT512 = T // 512  # 4
NCC = C // 128    # 4
SCALE = float(1.0 / np.sqrt(128.0))
SGRP = 2          # score tiles (512 wide) per exp group / PSUM banks per buf

_CACHE = {}


def _build():
    nc = bacc.Bacc("TRN2", target_bir_lowering=False, debug=False, num_devices=8)
    x_d = nc.dram_tensor("x", [T, C], FP32, kind="ExternalInput")
    w_d = nc.dram_tensor("w", [4, 1], FP32, kind="ExternalInput")
    bq_d = nc.dram_tensor("bq", [4, HS, C], FP32, kind="ExternalInput")
    bk_d = nc.dram_tensor("bk", [4, HS, C], FP32, kind="ExternalInput")
    bv_d = nc.dram_tensor("bv", [4, HS, C], FP32, kind="ExternalInput")
    out_d = nc.dram_tensor("out", [T, 4 * HS], BF16, kind="ExternalOutput")

    with tile.TileContext(nc) as tc:
        _emit(nc, tc, x_d, w_d, bq_d, bk_d, bv_d, out_d)
    nc.compile()
    return nc


def _emit(nc, tc, x_d, w_d, bq_d, bk_d, bv_d, out_d):
    from contextlib import ExitStack

    ctx = ExitStack()
    with ctx:
        # ---- persistent SBUF pools ----
        const_p = ctx.enter_context(tc.tile_pool(name="const", bufs=1))
        wall_p = ctx.enter_context(tc.tile_pool(name="wall", bufs=1))
        wts_p = ctx.enter_context(tc.tile_pool(name="wts", bufs=1))
        xall_p = ctx.enter_context(tc.tile_pool(name="xall", bufs=1))
        xt_p = ctx.enter_context(tc.tile_pool(name="xt", bufs=1))
        qk_p = ctx.enter_context(tc.tile_pool(name="qk", bufs=1))
        v_p = ctx.enter_context(tc.tile_pool(name="v", bufs=1))
        pt_p = ctx.enter_context(tc.tile_pool(name="pt", bufs=1))
        osb_p = ctx.enter_context(tc.tile_pool(name="osb", bufs=8))
        o_p = ctx.enter_context(tc.tile_pool(name="o", bufs=4))
        stage_p = ctx.enter_context(tc.tile_pool(name="stage", bufs=2))
        # ---- PSUM: 2 bufs x 2 banks for score groups, 2 x AV-chain bufs,
        # 2 x 1 bank for projections/transposes (separate pools so an AV
        # chain never waits on a projection's PSUM->SBUF cast)
        sps = ctx.enter_context(tc.tile_pool(name="sps", bufs=2, space="PSUM"))
        pav = ctx.enter_context(tc.tile_pool(name="pav", bufs=2, space="PSUM"))
        ps = ctx.enter_context(tc.tile_pool(name="ps", bufs=2, space="PSUM"))

        # ================= constants (first: Pool/DVE free, no DMA deps) ===
        ones_b = const_p.tile([128, C], BF16, tag="ones_b")
        nc.vector.memset(ones_b[:], 1.0)
        ident_b = const_p.tile([128, 128], BF16, tag="ident_b")
        nc.gpsimd.affine_select(
            ident_b[:], ones_b[:, 0:128], pattern=[[1, 128]],
            compare_op=ALU.is_equal, fill=0.0, base=0, channel_multiplier=-1)
        ident_f = const_p.tile([128, 128], FP32, tag="ident_f")
        nc.vector.tensor_copy(ident_f[:], ident_b[:])
        # causal triangle x4: tri4[:, r*128+t] = (t >= s) for r = 0..3
        tri4 = const_p.tile([128, 512], BF16, tag="tri4")
        for r in range(4):
            nc.gpsimd.affine_select(
                tri4[:, r * 128:(r + 1) * 128], ones_b[:, 0:128],
                pattern=[[1, 128]], compare_op=ALU.is_ge, fill=0.0, base=0,
                channel_multiplier=-1)
        tri3 = tri4[:].rearrange("p (r q) -> p r q", r=4)
        # start the PE streaming immediately (HAM clock-gate warm-up)
        for _ in range(3):
            sp0 = ps.tile([128, 512], FP32, name="ps", tag="ps")
            nc.tensor.matmul(sp0[:], ones_b[:, 0:128], ones_b[:],
                             start=True, stop=True)

        # ================= DMA issue (transfers run in background) =========
        # Three DMA rings in parallel so the startup critical path is
        # max(x chunk 0, q0/k0 bases) instead of their sum:
        #   sync:   x (first 4 tiles singly for early readiness, then groups),
        #           late v bases
        #   pool:   w_row + near-term weight bases
        #   scalar: light-unit q/k bases (ACT idle until first exp)
        w_row = const_p.tile([1, 4], FP32, tag="w_row")
        wall = [wall_p.tile([128, C], FP32, name=f"wall{j}", tag=f"wall{j}") for j in range(9)]
        xall = xall_p.tile([128, NT128 * C], FP32, tag="xall")
        xall3 = xall[:].rearrange("p (i c) -> p i c", c=C)
        x16 = x_d.ap().rearrange("(i p) c -> p i c", p=128)
        for tt in range(4):
            nc.sync.dma_start(xall3[:, tt:tt + 1, :], x16[:, tt:tt + 1, :])
        for grp in range(1, 4):
            nc.sync.dma_start(xall3[:, grp * 4:(grp + 1) * 4, :],
                              x16[:, grp * 4:(grp + 1) * 4, :])
        # j = 0..2 -> q(h0,h1,light), 3..5 -> k, 6..8 -> v; light packs
        # head2[0:64] + head3[0:64].
        nc.gpsimd.dma_start(w_row[:], w_d.ap().rearrange("a b -> b a"))
        nc.gpsimd.dma_start(wall[0][:], bq_d.ap()[0])
        nc.gpsimd.dma_start(wall[3][:], bk_d.ap()[0])
        nc.gpsimd.dma_start(wall[6][:], bv_d.ap()[0])
        nc.gpsimd.dma_start(wall[1][:], bq_d.ap()[1])
        nc.gpsimd.dma_start(wall[4][:], bk_d.ap()[1])
        nc.scalar.dma_start(wall[2][0:64, :], bq_d.ap()[2][0:64, :])
        nc.scalar.dma_start(wall[2][64:128, :], bq_d.ap()[3][0:64, :])
        nc.scalar.dma_start(wall[5][0:64, :], bk_d.ap()[2][0:64, :])
        nc.scalar.dma_start(wall[5][64:128, :], bk_d.ap()[3][0:64, :])
        nc.sync.dma_start(wall[7][:], bv_d.ap()[1])
        nc.sync.dma_start(wall[8][0:64, :], bv_d.ap()[2][0:64, :])
        nc.sync.dma_start(wall[8][64:128, :], bv_d.ap()[3][0:64, :])

        # ================= eff patterns (rank-1 K=1 matmuls) ===============
        # effA[d,e] = sum_i w_i * (d < hs_i) * (e < emb_i); light variant
        # effB over configs {1,3} uses the (d%64 < hs) pattern. All row
        # patterns live on partition 0 (engine partition-offset rule) in two
        # consolidated tiles: u4 blocks 0-3 heavy, 4-5 light; vrow blocks
        # emb=256 / emb=512.
        effA = const_p.tile([128, C], FP32, tag="effA")
        effB = const_p.tile([128, C], FP32, tag="effB")
        u4 = stage_p.tile([1, 6 * 128], BF16, name="u4", tag="u4", bufs=1)
        nc.vector.memset(u4[:], 0.0)
        nc.vector.memset(u4[0:1, 0:64], 1.0)        # cfg0 hs=64
        nc.vector.memset(u4[0:1, 128:160], 1.0)    # cfg1 hs=32
        nc.vector.memset(u4[0:1, 256:384], 1.0)    # cfg2 hs=128
        nc.vector.memset(u4[0:1, 384:448], 1.0)    # cfg3 hs=64
        nc.vector.memset(u4[0:1, 512:544], 1.0)    # light cfg1: d%64<32
        nc.vector.memset(u4[0:1, 576:608], 1.0)
        nc.vector.memset(u4[0:1, 640:768], 1.0)    # light cfg3: all
        vrow = stage_p.tile([1, 2 * C], BF16, name="vrow", tag="vrow", bufs=1)
        nc.vector.memset(vrow[:], 0.0)
        nc.vector.memset(vrow[0:1, 0:256], 1.0)    # emb=256 pattern
        nc.vector.memset(vrow[0:1, C:2 * C], 1.0)  # emb=512 pattern
        uw4 = stage_p.tile([1, 6 * 128], BF16, name="uw4", tag="uw4", bufs=1)
        for i in range(6):
            wi = (0, 1, 2, 3, 1, 3)[i]
            nc.vector.tensor_scalar_mul(
                uw4[0:1, i * 128:(i + 1) * 128],
                u4[0:1, i * 128:(i + 1) * 128], w_row[0:1, wi:wi + 1])
        EMBSEL = (0, 0, 1, 1, 0, 1)  # which vrow block per config
        pE = ps.tile([128, 512], FP32, name="ps", tag="ps")
        for n, i in enumerate((0, 1, 2, 3)):
            nc.tensor.matmul(pE[:], uw4[0:1, i * 128:(i + 1) * 128],
                             vrow[0:1, EMBSEL[i] * C:(EMBSEL[i] + 1) * C],
                             start=(n == 0), stop=(n == 3))
        nc.vector.tensor_copy(effA[:], pE[:])
        pE2 = ps.tile([128, 512], FP32, name="ps", tag="ps")
        for n, i in enumerate((4, 5)):
            nc.tensor.matmul(pE2[:], uw4[0:1, i * 128:(i + 1) * 128],
                             vrow[0:1, EMBSEL[i] * C:(EMBSEL[i] + 1) * C],
                             start=(n == 0), stop=(n == 1))
        nc.vector.tensor_copy(effB[:], pE2[:])

        # ================= persistent compute tensors ====================
        xt_all = xt_p.tile([128, NCC * T], BF16, tag="xt_all")
        xt = [xt_all[:, cc * T:(cc + 1) * T] for cc in range(NCC)]
        xt3 = xt_all[:].rearrange("p (a t) -> p a t", a=NCC)

        wtt9 = wts_p.tile([128, 9 * 512], BF16, tag="wtt9")
        wtt = [wtt9[:, j * 512:(j + 1) * 512] for j in range(9)]
        wt = [[wtt9[:, j * 512 + cc * 128:j * 512 + (cc + 1) * 128]
               for cc in range(NCC)] for j in range(9)]
        wpair3 = wtt9[:].rearrange("p (j q) -> p j q", q=512)

        qt = [qk_p.tile([128, T], BF16, name=f"qt{h}", tag=f"qt{h}") for h in range(3)]
        kt = [qk_p.tile([128, T], BF16, name=f"kt{h}", tag=f"kt{h}") for h in range(3)]
        # vall[h]: 16 s-chunks of 132 cols. heavy: [v(0:128) | ones(128)];
        # light packs [v_l0(0:64) | ones(64) | v_l1(65:129) | ones(129)].
        # ones columns are set once via strided memsets and never rewritten.
        vall = [v_p.tile([128, NT128 * 132], BF16, name=f"vall{h}", tag=f"vall{h}")
                for h in range(3)]
        vall3 = [vall[h][:].rearrange("p (i c) -> p i c", c=132) for h in range(3)]
        nc.gpsimd.memset(vall3[0][:, :, 128:129], 1.0)
        nc.gpsimd.memset(vall3[1][:, :, 128:129], 1.0)
        nc.gpsimd.memset(vall3[2][:, :, 64:65], 1.0)
        nc.gpsimd.memset(vall3[2][:, :, 129:130], 1.0)

        pts = [pt_p.tile([128, (4 * tj + 4) * 512], BF16, name=f"pt{tj}", tag=f"pt{tj}")
               for tj in range(NT512)]

        # (proj unit, light half, v lo, v hi, out col)
        AU = [
            (0, None, 0, 129, 0),
            (1, None, 0, 129, 128),
            (2, 0, 0, 65, 256),
            (2, 1, 65, 130, 384),
        ]

        def kt_ap(au_i, i):
            pu, half = AU[au_i][0], AU[au_i][1]
            sl = slice(i * 128, (i + 1) * 128)
            if half is None:
                return kt[pu][:, sl]
            return kt[2][64 * half:64 * half + 64, sl]

        def qt_ap(au_i, tj):
            pu, half = AU[au_i][0], AU[au_i][1]
            sl = slice(tj * 512, (tj + 1) * 512)
            if half is None:
                return qt[pu][:, sl]
            return qt[2][64 * half:64 * half + 64, sl]

        # ================= work items (filler queue) =====================
        def xprep(tt):
            def fn():
                # fp32 transpose straight off the DMA; the PSUM->SBUF
                # consolidation does the bf16 cast. During startup ACT is
                # idle, so the first four consolidations run there instead
                # of on the (startup-critical) DVE.
                ptp = ps.tile([128, 512], FP32, name="ps", tag="ps")
                for cc in range(NCC):
                    nc.tensor.transpose(
                        ptp[:, cc * 128:(cc + 1) * 128],
                        xall[:, tt * C + cc * 128:tt * C + (cc + 1) * 128],
                        ident_f[:])
                nc.vector.tensor_copy(
                    xt3[:, :, tt * 128:(tt + 1) * 128],
                    ptp[:].rearrange("p (a t) -> p a t", a=NCC))
            return (700, fn)

        def witem(j):
            def fn():
                w_bf = stage_p.tile([128, C], BF16, name="w_bf", tag="w_bf", bufs=2)
                nc.vector.tensor_mul(w_bf[:], wall[j][:],
                                     effB[:] if (j % 3) == 2 else effA[:])
                ptp = ps.tile([128, 512], BF16, name="ps", tag="ps")
                for cc in range(NCC):
                    nc.tensor.transpose(
                        ptp[:, cc * 128:(cc + 1) * 128],
                        w_bf[:, cc * 128:(cc + 1) * 128], ident_b[:])
                cp = nc.scalar.copy if j in (0, 3) else nc.vector.tensor_copy
                cp(wtt[j], ptp[:])
            return (400, fn)

        def qkitem(pu, tj, which):  # which: 0 = q, 1 = k
            def fn():
                j = 3 * which + pu
                dst = (qt if which == 0 else kt)[pu]
                p = ps.tile([128, 512], FP32, name="ps", tag="ps")
                for cc in range(NCC):
                    nc.tensor.matmul(
                        p[:], wt[j][cc][:], xt[cc][:, tj * 512:(tj + 1) * 512],
                        start=(cc == 0), stop=(cc == NCC - 1))
                cp = (nc.scalar.copy if (pu == 0 and tj == 0)
                      else nc.vector.tensor_copy)
                cp(dst[:, tj * 512:(tj + 1) * 512], p[:])
            return (1000, fn)

        def vitem(pu, i):
            def fn():
                p = ps.tile([128, 512], FP32, name="ps", tag="ps")
                if pu == 0:
                    # both heavy heads' v in one 256-col moving pass
                    for cc in range(NCC):
                        nc.tensor.matmul(
                            p[:, 0:256], xt[cc][:, i * 128:(i + 1) * 128],
                            wpair3[:, 6:8, cc * 128:(cc + 1) * 128],
                            start=(cc == 0), stop=(cc == NCC - 1))
                    nc.vector.tensor_copy(
                        vall[0][:, i * 132:i * 132 + 128], p[:, 0:128])
                    nc.vector.tensor_copy(
                        vall[1][:, i * 132:i * 132 + 128], p[:, 128:256])
                else:
                    for cc in range(NCC):
                        nc.tensor.matmul(
                            p[:, 0:128], xt[cc][:, i * 128:(i + 1) * 128],
                            wt[6 + pu][cc][:],
                            start=(cc == 0), stop=(cc == NCC - 1))
                    vt_i = vall[pu][:, i * 132:(i + 1) * 132]
                    nc.vector.tensor_copy(vt_i[:, 0:64], p[:, 0:64])
                    nc.vector.tensor_copy(vt_i[:, 65:129], p[:, 64:128])
            return (550 if pu == 0 else 350, fn)

        # per-(unit, t-chunk) output stage: 4 normalized m-blocks gathered
        # into one SBUF tile, stored with a single strided DMA.
        out16 = out_d.ap().rearrange("(i p) c -> p i c", p=128)
        ob4_cur = {}

        def avitem(au_i, tj, m):
            pu, half, v_lo, v_hi, ocol = AU[au_i]
            w = v_hi - v_lo
            wm = w - 1
            ti = 4 * tj + m

            def fn():
                op = pav.tile([128, 256], FP32, name="pav", tag="pav")
                ptile = pts[tj]
                for i in range(ti + 1):
                    nc.tensor.matmul(
                        op[:, 0:w],
                        ptile[:, i * 512 + m * 128: i * 512 + (m + 1) * 128],
                        vall[pu][:, i * 132 + v_lo:i * 132 + v_hi],
                        start=(i == 0), stop=(i == ti))
                osb = osb_p.tile([128, 129], FP32, name="osb", tag="osb")
                nc.vector.tensor_copy(osb[:, 0:w], op[:, 0:w])
                if m == 0:
                    tag = "ob4h" if wm == 128 else "ob4l"
                    ob4_cur[au_i] = o_p.tile([128, 4 * wm], BF16,
                                             name=tag, tag=tag, bufs=3)
                ob4 = ob4_cur[au_i]
                nc.gpsimd.normalize_recip(
                    ob4[:, m * wm:(m + 1) * wm], osb[:, 0:wm], osb[:, wm:w])
                if m == 3:
                    nc.sync.dma_start(
                        out16[:, 4 * tj:4 * tj + 4, ocol:ocol + wm],
                        ob4[:].rearrange("p (m c) -> p m c", m=4))
            return ((ti + 1) * w * 0.46 + 250, fn)

        def spinitem():
            def fn():
                sp0 = ps.tile([128, 512], FP32, name="ps", tag="ps")
                nc.tensor.matmul(sp0[:], ones_b[:, 0:128], ones_b[:],
                                 start=True, stop=True)
            return (215, fn)

        # ---- filler queue: emitted into PE gaps while ACT drains exp ----
        # Items are (deadline_chunk, cost_ns, fn). FIFO; deadlines force
        # emission before the consumer chunk starts (Tile resolves data deps
        # by emission order, so a producer must be emitted before its reader).
        fill = []

        def drain(budget):
            spent = 0.0
            while fill and spent < budget:
                _, cost, fn = fill.pop(0)
                fn()
                spent += cost

        def drain_due(ci):
            # emit every due item, preserving relative queue order (producer
            # deadlines are always <= their consumers', so this is dep-safe)
            due = [it for it in fill if it[0] <= ci]
            if due:
                fill[:] = [it for it in fill if it[0] > ci]
                for it in due:
                    it[2]()

        def drain_all():
            while fill:
                fill.pop(0)[2]()

        # ================= score chunks with budgeted filler ==============
        def sched_scores(au_i, tj, final=False):
            S = 4 * tj + 4
            ptile = pts[tj]
            g = 0
            done_av = 0
            while g < S:
                gw = min(SGRP, S - g)
                sp = sps.tile([128, SGRP * 512], FP32, name="sps", tag="sps")
                for k in range(gw):
                    i = g + k
                    # diagonal chunks (i = 4tj+r): t-cols below 128r are
                    # strictly above the causal boundary and never read
                    # downstream — skip them in the matmul (exp still runs
                    # full-width over stale PSUM there; harmless, unread)
                    lo = max(0, (i - 4 * tj) * 128)
                    qa = qt_ap(au_i, tj)
                    nc.tensor.matmul(sp[:, k * 512 + lo:(k + 1) * 512],
                                     kt_ap(au_i, i), qa[:, lo:512],
                                     start=True, stop=True)
                nc.scalar.activation(
                    ptile[:, g * 512:(g + gw) * 512],
                    sp[:, 0:gw * 512], AF.Exp, scale=SCALE)
                g += gw
                if final:
                    # last chunk (tj == 0): mask + AV per diagonal block as
                    # soon as its exp lands, so the tail is a single m-block
                    while done_av < min(g, 4):
                        r = done_av
                        blk = ptile[:, r * 512 + r * 128:
                                    r * 512 + (r + 1) * 128]
                        nc.vector.tensor_mul(
                            blk, blk, tri4[:, r * 128:(r + 1) * 128])
                        avitem(au_i, tj, r)[1]()
                        done_av += 1
                else:
                    # ACT time of this group minus our own PE time
                    drain(gw * 427 + 470 - gw * 240)
            if not final:
                # causal tri on the 4 diagonal 128-blocks, one fused op:
                # block r sits at 128-col index 16*tj + 5*r (stride 5)
                dst = ptile[:].rearrange("p (i q) -> p i q", q=128)[
                    :, 16 * tj:16 * tj + 16:5, :]
                nc.vector.tensor_mul(dst, dst, tri3)

        # ================= main pipeline ==================================
        # HAM warm-up: keep PE streaming through startup DMA waits so the
        # clock gate reaches 8/8 before the first score chunk.
        def spin(n):
            for _ in range(n):
                sp0 = ps.tile([128, 512], FP32, name="ps", tag="ps")
                nc.tensor.matmul(sp0[:], ones_b[:, 0:128], ones_b[:],
                                 start=True, stop=True)

        # startup: weights q0/k0, x chunk 0, unit-0 qk proj for t-chunk 0
        spin(4)
        witem(0)[1]()
        witem(3)[1]()
        spin(4)
        for tt in range(4):
            xprep(tt)[1]()
        spin(2)
        qkitem(0, 0, 0)[1]()
        qkitem(0, 0, 1)[1]()

        # per-chunk filler: startup leftovers + next-unit projections + AV.
        # E(dl, item) attaches the mandatory-emission deadline chunk.
        def E(dl, item):
            return (dl, item[0], item[1])

        extras = {c: [] for c in range(16)}
        for _ in range(3):
            extras[0].append(E(99, spinitem()))
            extras[1].append(E(99, spinitem()))
        extras[0] += [E(1, witem(6)),
                      E(1, vitem(0, 0)), E(1, vitem(0, 1)),
                      E(1, vitem(0, 2)), E(1, vitem(0, 3)),
                      E(1, xprep(4)), E(1, xprep(5)),
                      E(1, xprep(6)), E(1, xprep(7)),
                      E(1, qkitem(0, 1, 0)), E(1, qkitem(0, 1, 1))]
        extras[1] += [E(2, xprep(8)), E(2, xprep(9)),
                      E(2, xprep(10)), E(2, xprep(11)),
                      E(2, qkitem(0, 2, 0)), E(2, qkitem(0, 2, 1)),
                      E(2, vitem(0, 4)), E(2, vitem(0, 5)),
                      E(2, vitem(0, 6)), E(2, vitem(0, 7)),
                      E(4, witem(1)), E(4, witem(4))]
        extras[2] += [E(3, xprep(12)), E(3, xprep(13)),
                      E(3, xprep(14)), E(3, xprep(15)),
                      E(3, qkitem(0, 3, 0)), E(3, qkitem(0, 3, 1)),
                      E(3, vitem(0, 8)), E(3, vitem(0, 9)),
                      E(3, vitem(0, 10)), E(3, vitem(0, 11)),
                      E(5, witem(7))]
        extras[3] += [E(4, vitem(0, 12)), E(4, vitem(0, 13)),
                      E(4, vitem(0, 14)), E(4, vitem(0, 15)),
                      E(8, witem(2)), E(8, witem(5)), E(8, witem(8))]
        # unit-0 windows carry proj of unit 1; unit-1 windows carry the
        # packed-light projections (units 2 and 3 share proj unit 2).
        for u in range(2):
            pu = u + 1
            base_c = 4 * pu
            for tj in range(NT512):
                c = 4 * u + tj
                extras[c] += [E(base_c + tj, qkitem(pu, tj, 0)),
                              E(base_c + tj, qkitem(pu, tj, 1))]
                if pu == 2:  # light v; heavy v pair is computed with pu=0
                    extras[c] += [E(base_c + 1 + tj, vitem(pu, 4 * tj + r))
                                  for r in range(4)]

        # unit 3 runs [2,3,1,0] so the tail AV after the last scores chunk
        # is the small tj=0 one (10 matmuls) instead of tj=3 (58).
        CH = ([(u, t) for u in range(3) for t in range(4)]
              + [(3, 2), (3, 3), (3, 1), (3, 0)])

        def av_deadline(ci, ptj):
            # AV of (pau, ptj) must be emitted before exp next rewrites
            # pts[ptj]; drain_due runs before sched_scores, so deadline may
            # equal that chunk's index.
            for c2 in range(ci, len(CH)):
                if CH[c2][1] == ptj:
                    return min(ci + 1, c2)
            return ci + 1

        for ci, (au_i, tj) in enumerate(CH):
            last = ci == len(CH) - 1
            if ci > 0:
                pau, ptj = CH[ci - 1]
                dl = ci if last else av_deadline(ci, ptj)
                for m in range(4):
                    fill.append(E(dl, avitem(pau, ptj, m)))
            fill.extend(extras[ci])
            drain_due(ci)
            sched_scores(au_i, tj, final=last)
        drain_all()


def _shard_inputs(x, weights, base_K, base_Q, base_V):
    in_maps = []
    for c in range(8):
        b = c // 2
        hsel = [0, 1, 4, 5] if c % 2 == 0 else [2, 3, 6, 7]
        in_maps.append({
            "x": np.ascontiguousarray(x[b]),
            "w": np.ascontiguousarray(weights.reshape(4, 1)),
            "bq": np.ascontiguousarray(base_Q[hsel]),
            "bk": np.ascontiguousarray(base_K[hsel]),
            "bv": np.ascontiguousarray(base_V[hsel]),
        })
    return in_maps


def _gather(results):
    out = np.zeros((4, T, 8 * HS), np.float32)
    for c in range(8):
        o = results[c]["out"]
        hsel = [0, 1, 4, 5] if c % 2 == 0 else [2, 3, 6, 7]
        for j, h in enumerate(hsel):
            out[c // 2][:, h * HS:(h + 1) * HS] = o[:, j * HS:(j + 1) * HS]
    return out


def get_nc():
    if "nc" not in _CACHE:
        _CACHE["nc"] = _build()
    return _CACHE["nc"]


def kernel(x, weights, base_K, base_Q, base_V):
    x = np.asarray(x, np.float32)
    weights = np.asarray(weights, np.float32)
    base_K = np.asarray(base_K, np.float32)
    base_Q = np.asarray(base_Q, np.float32)
    base_V = np.asarray(base_V, np.float32)
    nc = get_nc()
    in_maps = _shard_inputs(x, weights, base_K, base_Q, base_V)
    res = run_bass_kernel_spmd(nc, in_maps, core_ids=list(range(8)))
    return _gather(res.results)
